# revision 3
# baseline (speedup 1.0000x reference)
"""GAT (3-layer, PyG-style) on 8 Trainium2 NeuronCores via Bass/Tile.

Sharding: edges are partitioned by dst-node range (1250 nodes per core).
Each layer: (A) node-parallel feature projection + attention logits,
AllGather of the node-major row table [hx | al_src | pad] into each
core's DRAM (the al_dst table stays core-local); (B) edge-parallel
aggregation: hardware dma_gather of per-edge rows by src id (features +
src logit) and by local dst id (dst logit); exp(leaky(logits)) scaling;
segment-sum via one-hot-mask matmuls accumulated in PSUM (the softmax
denominator rides a second matmul; the division is applied per dst node
afterwards). The softmax max-subtraction is skipped: logits are bounded
by construction (|e| < ~15) so exp stays well inside fp32 range.
"""

import sys

sys.path.insert(0, "/opt/trn_rl_repo")

import numpy as np

N = 10000
E = 160000
NCORES = 8
NS = 1250          # nodes per core
T = 10             # dst tiles per core
TS = 125           # nodes per dst tile
NEG = 0.2          # leaky_relu slope

F_IN = 256
HC = 512
NUM_CLASSES = 40

_cache = {}


def _wrap16(a):
    """[T, CH, 128] per-slot values -> [128, T*CH*8] int16 wrapped layout.

    dma_gather consumes index i from (partition i%16, col i//16), replicated
    across the eight 16-partition groups. Slot (t, j, q) is flat index
    i = j*128 + q within tile t's column block.
    """
    Tn, CHn, _ = a.shape
    b = a.reshape(Tn * CHn * 8, 16).astype(np.int16)
    m = np.ascontiguousarray(b.T)          # [16, T*CH*8]
    return np.tile(m, (8, 1))              # [128, T*CH*8]


def _preprocess(edge_index):
    """Group edges (incl. self-loops) by (core, dst-tile); pad chunks of 128.

    Padding slots keep src=0 / dst_local=0 (harmless gathers) and get
    mask value 127 (>= TS) so the one-hot mask row is all-zero and they
    contribute nothing to sums.
    """
    src = np.concatenate([np.asarray(edge_index[0]), np.arange(N)]).astype(np.int64)
    dst = np.concatenate([np.asarray(edge_index[1]), np.arange(N)]).astype(np.int64)
    gtile = dst // TS                       # global tile id 0..79
    order = np.argsort(gtile, kind="stable")
    src, dst, gtile = src[order], dst[order], gtile[order]
    counts = np.bincount(gtile, minlength=NCORES * T)
    CH = int(np.ceil(counts.max() / 128))
    starts = np.concatenate([[0], np.cumsum(counts)])

    idx16s = np.empty((NCORES, 128, T * CH * 8), np.int16)
    idx16d = np.empty((NCORES, 128, T * CH * 8), np.int16)
    dl_f32 = np.empty((NCORES, 128, T * CH), np.float32)
    for k in range(NCORES):
        sa = np.zeros((T, CH, 128), np.int64)
        da = np.zeros((T, CH, 128), np.int64)
        dl = np.full((T, CH, 128), 127.0, np.float32)
        for t in range(T):
            g = k * T + t
            s0, s1 = starts[g], starts[g + 1]
            m = s1 - s0
            i = np.arange(m)
            js, qs = i // 128, i % 128
            sa[t, js, qs] = src[s0:s1]
            da[t, js, qs] = dst[s0:s1] - k * NS
            dl[t, js, qs] = (dst[s0:s1] % TS).astype(np.float32)
        idx16s[k] = _wrap16(sa)
        idx16d[k] = _wrap16(da)
        # mask-build layout: value at (p, t*CH + j) = dl[t, j, p]
        dl_f32[k] = dl.transpose(2, 0, 1).reshape(128, T * CH)
    return CH, idx16s, idx16d, dl_f32


def _feat_major(w, fb):
    """[K, M] -> [128, fb, M] with element (p, b, m) = w[b*128 + p, m]."""
    K, M = w.shape
    assert K == fb * 128
    return np.ascontiguousarray(w.reshape(fb, 128, M).transpose(1, 0, 2))


def _block_diag_a(a_src, a_dst):
    """[H, C] pair -> [H*C, 16] block-diag (cols 0:8 src, 8:16 dst)."""
    h, c = a_src.shape
    blk = np.zeros((h * c, 16), np.float32)
    for i in range(h):
        blk[i * c : (i + 1) * c, i] = a_src[i]
        blk[i * c : (i + 1) * c, 8 + i] = a_dst[i]
    return blk


def _build_program(CH):
    import concourse.bass as bass
    import concourse.mybir as mybir
    import concourse.bacc as bacc
    import concourse.tile as tile

    f32 = mybir.dt.float32
    i16 = mybir.dt.int16
    Alu = mybir.AluOpType
    Act = mybir.ActivationFunctionType

    nc = bacc.Bacc(
        "TRN2",
        target_bir_lowering=False,
        debug=False,
        enable_asserts=False,
        num_devices=NCORES,
    )

    icols = T * CH * 8
    NSL = [(0, 512), (512, 512), (1024, 226)]  # 1250 split for matmul N<=512

    # layer specs: (fin_blocks, fout, fout_blocks, H, C, row)
    # row: gathered-table row length in f32 (256B-aligned for dma_gather)
    specs = [
        (2, 512, 4, 8, 64, 576),
        (4, 512, 4, 8, 64, 576),
        (4, 40, 1, 1, 40, 64),
    ]
    ALROW = 64  # local al_dst table row (f32)

    # ---- external I/O ----
    xt_in = nc.dram_tensor("xt", [128, 2, NS], f32, kind="ExternalInput")
    idxs_in = nc.dram_tensor("idx16s", [128, icols], i16, kind="ExternalInput")
    idxd_in = nc.dram_tensor("idx16d", [128, icols], i16, kind="ExternalInput")
    dstl_in = nc.dram_tensor("dstl", [128, T * CH], f32, kind="ExternalInput")
    w_in = [
        nc.dram_tensor("W0", [128, 2, 512], f32, kind="ExternalInput"),
        nc.dram_tensor("W1", [128, 4, 512], f32, kind="ExternalInput"),
        nc.dram_tensor("W2", [128, 4, 40], f32, kind="ExternalInput"),
    ]
    a_in = [
        nc.dram_tensor("A0", [128, 4, 16], f32, kind="ExternalInput"),
        nc.dram_tensor("A1", [128, 4, 16], f32, kind="ExternalInput"),
        nc.dram_tensor("A2", [40, 2], f32, kind="ExternalInput"),
    ]
    b_in = [
        nc.dram_tensor("B0", [128, 512], f32, kind="ExternalInput"),
        nc.dram_tensor("B1", [128, 512], f32, kind="ExternalInput"),
        nc.dram_tensor("B2", [128, 40], f32, kind="ExternalInput"),
    ]
    iota_in = nc.dram_tensor("iota", [128, 128], f32, kind="ExternalInput")
    ident_in = nc.dram_tensor("ident", [128, 128], f32, kind="ExternalInput")
    out_ext = nc.dram_tensor("out", [NS, NUM_CLASSES], f32, kind="ExternalOutput")

    # internal DRAM
    agin = [
        nc.dram_tensor(f"agin{l}", [NS, specs[l][5]], f32, kind="Internal")
        for l in range(3)
    ]
    tbl = [
        nc.dram_tensor(
            f"tbl{l}", [N, specs[l][5]], f32, kind="Internal", addr_space="Shared"
        )
        for l in range(3)
    ]
    aldloc = [
        nc.dram_tensor(f"ald{l}", [NS, ALROW], f32, kind="Internal")
        for l in range(3)
    ]

    groups = [list(range(NCORES))]

    with tile.TileContext(nc) as tc:
        with (
            tc.tile_pool(name="const", bufs=1) as cp,
            tc.tile_pool(name="sb", bufs=2) as sb,
            tc.tile_pool(name="persist", bufs=1) as pp,
            tc.tile_pool(name="psA", bufs=2, space="PSUM") as psA,
            tc.tile_pool(name="psB", bufs=2, space="PSUM") as psB,
        ):
            # ---- load constants ----
            def load_const(t_in, shape, dtype=f32):
                t = cp.tile(shape, dtype, tag=t_in.name)
                nc.sync.dma_start(out=t[:], in_=t_in[:])
                return t

            xt0 = pp.tile([128, 2, NS], f32, tag="xt0")
            nc.sync.dma_start(out=xt0[:], in_=xt_in[:])
            idx16s = load_const(idxs_in, [128, icols], i16)
            idx16d = load_const(idxd_in, [128, icols], i16)
            dstl = load_const(dstl_in, [128, T * CH], f32)
            Wt = [
                load_const(w_in[0], [128, 2, 512]),
                load_const(w_in[1], [128, 4, 512]),
                load_const(w_in[2], [128, 4, 40]),
            ]
            At = [
                load_const(a_in[0], [128, 4, 16]),
                load_const(a_in[1], [128, 4, 16]),
                load_const(a_in[2], [40, 2]),
            ]
            Bt = [
                load_const(b_in[0], [128, 512]),
                load_const(b_in[1], [128, 512]),
                load_const(b_in[2], [128, 40]),
            ]
            iota = load_const(iota_in, [128, 128])
            ident = load_const(ident_in, [128, 128])

            xt_cur = xt0
            for l, (fbi, fout, fbo, Hh, Cc, row) in enumerate(specs):
                # ============ phase A: hxT = W.T-contract(XT), logits ============
                if l < 2:
                    hxT = pp.tile([128, fbo, NS], f32, tag="hxT")
                else:
                    hxT = pp.tile([40, NS], f32, tag="hxT2")
                alT = pp.tile([16 if l < 2 else 2, NS], f32, tag="alT")

                for fo in range(fbo):
                    mpart = 128 if l < 2 else 40
                    for (n0, nw) in NSL:
                        ps = psA.tile([mpart, nw], f32, tag="big")
                        for fb in range(fbi):
                            nc.tensor.matmul(
                                out=ps[:],
                                lhsT=Wt[l][:, fb, fo * 128 : fo * 128 + mpart],
                                rhs=xt_cur[:, fb, n0 : n0 + nw],
                                start=(fb == 0),
                                stop=(fb == fbi - 1),
                            )
                        if l < 2:
                            nc.vector.tensor_copy(
                                out=hxT[:, fo, n0 : n0 + nw], in_=ps[:]
                            )
                        else:
                            nc.vector.tensor_copy(out=hxT[:, n0 : n0 + nw], in_=ps[:])

                # attention logits alT = A.T @ hxT   (contract over fout)
                napart = 16 if l < 2 else 2
                for (n0, nw) in NSL:
                    ps = psA.tile([napart, nw], f32, tag="big")
                    if l < 2:
                        for fb in range(fbo):
                            nc.tensor.matmul(
                                out=ps[:],
                                lhsT=At[l][:, fb, :napart],
                                rhs=hxT[:, fb, n0 : n0 + nw],
                                start=(fb == 0),
                                stop=(fb == fbo - 1),
                            )
                    else:
                        nc.tensor.matmul(
                            out=ps[:],
                            lhsT=At[l][:40, :2],
                            rhs=hxT[:40, n0 : n0 + nw],
                            start=True,
                            stop=True,
                        )
                    nc.vector.tensor_copy(out=alT[:, n0 : n0 + nw], in_=ps[:])

                # transpose to node-major rows; stage big table + local ald table
                for t in range(T):
                    stg = sb.tile([TS, row], f32, tag="stg")
                    stal = sb.tile([TS, ALROW], f32, tag="stal")
                    nsl = slice(t * TS, (t + 1) * TS)
                    if l < 2:
                        for fo in range(fbo):
                            pt = psA.tile([TS, 128], f32, tag="tr")
                            nc.tensor.transpose(
                                out=pt[:], in_=hxT[:, fo, nsl], identity=ident[:]
                            )
                            nc.vector.tensor_copy(
                                out=stg[:, fo * 128 : (fo + 1) * 128], in_=pt[:]
                            )
                        pa = psA.tile([TS, 16], f32, tag="tr")
                        nc.tensor.transpose(
                            out=pa[:], in_=alT[:16, nsl], identity=ident[:16, :16]
                        )
                        nc.vector.tensor_copy(out=stg[:, 512:528], in_=pa[:, 0:16])
                        nc.vector.memset(stg[:, 528:row], 0.0)
                        nc.vector.memset(stal[:, 8:ALROW], 0.0)
                        nc.vector.tensor_copy(out=stal[:, 0:8], in_=pa[:, 8:16])
                    else:
                        pt = psA.tile([TS, 40], f32, tag="tr")
                        nc.tensor.transpose(
                            out=pt[:], in_=hxT[:40, nsl], identity=ident[:40, :40]
                        )
                        nc.vector.tensor_copy(out=stg[:, 0:40], in_=pt[:])
                        pa = psA.tile([TS, 2], f32, tag="tr")
                        nc.tensor.transpose(
                            out=pa[:], in_=alT[:2, nsl], identity=ident[:2, :2]
                        )
                        nc.vector.tensor_copy(out=stg[:, 40:41], in_=pa[:, 0:1])
                        nc.vector.memset(stg[:, 41:row], 0.0)
                        nc.vector.memset(stal[:, 1:ALROW], 0.0)
                        nc.vector.tensor_copy(out=stal[:, 0:1], in_=pa[:, 1:2])
                    nc.sync.dma_start(out=agin[l][nsl, :], in_=stg[:])
                    nc.sync.dma_start(out=aldloc[l][nsl, :], in_=stal[:])

                nc.gpsimd.collective_compute(
                    "AllGather",
                    Alu.bypass,
                    replica_groups=groups,
                    ins=[agin[l][:]],
                    outs=[tbl[l][:]],
                )

                # ============ phase B: edge aggregation ============
                if l < 2:
                    xt_next = pp.tile([128, 4, NS], f32, tag=f"xt{l + 1}")
                hc0 = max(1, (CH + 2) // 3)  # chunks per gather batch
                for t in range(T):
                    ps_out = psB.tile([TS, fout], f32, tag="out")
                    ps_den = psB.tile([TS, Hh], f32, tag="den")
                    jj = 0
                    while jj < CH:
                        hc = min(hc0, CH - jj)
                        ic0 = (t * CH + jj) * 8
                        G = sb.tile([128, hc0, row], f32, tag="G")
                        nc.gpsimd.dma_gather(
                            out_ap=G[:, :hc, :],
                            in_ap=tbl[l][:],
                            idxs_ap=idx16s[:, ic0 : ic0 + hc * 8],
                            num_idxs=hc * 128,
                            num_idxs_reg=hc * 128,
                            elem_size=row,
                        )
                        Gd = sb.tile([128, hc0, ALROW], f32, tag="Gd")
                        nc.gpsimd.dma_gather(
                            out_ap=Gd[:, :hc, :],
                            in_ap=aldloc[l][:],
                            idxs_ap=idx16d[:, ic0 : ic0 + hc * 8],
                            num_idxs=hc * 128,
                            num_idxs_reg=hc * 128,
                            elem_size=ALROW,
                        )
                        # logits: leaky(als[src] + ald[dst]), then exp
                        lg = sb.tile([128, hc0, Hh], f32, tag="lg")
                        nc.vector.tensor_tensor(
                            out=lg[:, :hc, :],
                            in0=G[:, :hc, fout : fout + Hh],
                            in1=Gd[:, :hc, 0:Hh],
                            op=Alu.add,
                        )
                        t2 = sb.tile([128, hc0, Hh], f32, tag="t2")
                        nc.vector.tensor_scalar(
                            out=t2[:, :hc, :],
                            in0=lg[:, :hc, :],
                            scalar1=NEG,
                            scalar2=None,
                            op0=Alu.mult,
                        )
                        nc.vector.tensor_tensor(
                            out=lg[:, :hc, :],
                            in0=lg[:, :hc, :],
                            in1=t2[:, :hc, :],
                            op=Alu.max,
                        )
                        ex = sb.tile([128, hc0, Hh], f32, tag="ex")
                        nc.scalar.activation(
                            out=ex[:, :hc, :], in_=lg[:, :hc, :], func=Act.Exp
                        )
                        # scale features by exp(logit) per (edge, head)
                        G2 = sb.tile([128, hc0, Hh, Cc], f32, tag="G2")
                        nc.vector.tensor_tensor(
                            out=G2[:, :hc, :, :],
                            in0=G[:, :hc, 0:fout].rearrange(
                                "p j (h c) -> p j h c", h=Hh
                            ),
                            in1=ex[:, :hc, :].to_broadcast([128, hc, Hh, Cc]),
                            op=Alu.mult,
                        )
                        for j in range(hc):
                            mask = sb.tile([128, TS], f32, tag="mask")
                            nc.vector.tensor_scalar(
                                out=mask[:],
                                in0=iota[:, 0:TS],
                                scalar1=dstl[:, t * CH + jj + j : t * CH + jj + j + 1],
                                scalar2=None,
                                op0=Alu.is_equal,
                            )
                            glob_j = jj + j
                            nc.tensor.matmul(
                                out=ps_out[:],
                                lhsT=mask[:],
                                rhs=G2[:, j],
                                start=(glob_j == 0),
                                stop=(glob_j == CH - 1),
                                skip_group_check=True,
                            )
                            nc.tensor.matmul(
                                out=ps_den[:],
                                lhsT=mask[:],
                                rhs=ex[:, j],
                                start=(glob_j == 0),
                                stop=(glob_j == CH - 1),
                                skip_group_check=True,
                            )
                        jj += hc
                    # tail: divide by denominator, bias, activation
                    den = sb.tile([TS, Hh], f32, tag="dent")
                    nc.vector.tensor_scalar(
                        out=den[:], in0=ps_den[:], scalar1=1e-16, scalar2=None,
                        op0=Alu.add,
                    )
                    rcp = sb.tile([TS, Hh], f32, tag="rcp")
                    nc.vector.reciprocal(out=rcp[:], in_=den[:])
                    y = sb.tile([TS, fout], f32, tag="y")
                    nc.vector.tensor_tensor(
                        out=y[:].rearrange("p (h c) -> p h c", h=Hh),
                        in0=ps_out[:].rearrange("p (h c) -> p h c", h=Hh),
                        in1=rcp[:].to_broadcast([TS, Hh, Cc]),
                        op=Alu.mult,
                    )
                    nc.vector.tensor_tensor(
                        out=y[:], in0=y[:], in1=Bt[l][:TS, :fout], op=Alu.add
                    )
                    nsl = slice(t * TS, (t + 1) * TS)
                    if l < 2:
                        # elu(y) = max(y,0) + exp(min(y,0)) - 1
                        ymin = sb.tile([TS, fout], f32, tag="ymin")
                        nc.vector.tensor_scalar(
                            out=ymin[:], in0=y[:], scalar1=0.0, scalar2=None,
                            op0=Alu.min,
                        )
                        eneg = sb.tile([TS, fout], f32, tag="eneg")
                        nc.scalar.activation(out=eneg[:], in_=ymin[:], func=Act.Exp)
                        nc.vector.tensor_scalar(
                            out=eneg[:], in0=eneg[:], scalar1=1.0, scalar2=None,
                            op0=Alu.subtract,
                        )
                        x2 = sb.tile([TS, fout], f32, tag="x2")
                        nc.vector.tensor_scalar(
                            out=x2[:], in0=y[:], scalar1=0.0, scalar2=None,
                            op0=Alu.max,
                        )
                        nc.vector.tensor_tensor(
                            out=x2[:], in0=x2[:], in1=eneg[:], op=Alu.add
                        )
                        # transpose into next layer's feature-major XT
                        for fo in range(4):
                            pt = psA.tile([128, TS], f32, tag="tr")
                            nc.tensor.transpose(
                                out=pt[:],
                                in_=x2[:, fo * 128 : (fo + 1) * 128],
                                identity=ident[:TS, :TS],
                            )
                            nc.vector.tensor_copy(out=xt_next[:, fo, nsl], in_=pt[:])
                    else:
                        nc.sync.dma_start(out=out_ext[nsl, :], in_=y[:, :NUM_CLASSES])
                if l < 2:
                    xt_cur = xt_next

    nc.compile()
    return nc


def _make_in_maps(inputs, CH, idx16s, idx16d, dl_f32):
    x = np.asarray(inputs["x"], np.float32)
    iota = np.tile(np.arange(128, dtype=np.float32), (128, 1))
    ident = np.eye(128, dtype=np.float32)
    common = {
        "W0": _feat_major(np.asarray(inputs["W0"], np.float32), 2),
        "W1": _feat_major(np.asarray(inputs["W1"], np.float32), 4),
        "W2": _feat_major(np.asarray(inputs["W2"], np.float32), 4),
        "A0": _feat_major(
            _block_diag_a(np.asarray(inputs["a_src0"]), np.asarray(inputs["a_dst0"])), 4
        ),
        "A1": _feat_major(
            _block_diag_a(np.asarray(inputs["a_src1"]), np.asarray(inputs["a_dst1"])), 4
        ),
        "A2": np.ascontiguousarray(
            np.stack(
                [
                    np.asarray(inputs["a_src2"], np.float32)[0],
                    np.asarray(inputs["a_dst2"], np.float32)[0],
                ],
                axis=1,
            )
        ),
        "B0": np.tile(np.asarray(inputs["b0"], np.float32), (128, 1)),
        "B1": np.tile(np.asarray(inputs["b1"], np.float32), (128, 1)),
        "B2": np.tile(np.asarray(inputs["b2"], np.float32), (128, 1)),
        "iota": iota,
        "ident": ident,
    }
    in_maps = []
    for k in range(NCORES):
        xs = x[k * NS : (k + 1) * NS]  # [NS, 256]
        xt = np.ascontiguousarray(xs.T.reshape(2, 128, NS).transpose(1, 0, 2))
        in_maps.append(
            dict(
                common,
                xt=xt,
                idx16s=idx16s[k],
                idx16d=idx16d[k],
                dstl=dl_f32[k],
            )
        )
    return in_maps


def get_program_and_maps(inputs):
    CH, idx16s, idx16d, dl_f32 = _preprocess(np.asarray(inputs["edge_index"]))
    if CH not in _cache:
        _cache[CH] = _build_program(CH)
    nc = _cache[CH]
    return nc, _make_in_maps(inputs, CH, idx16s, idx16d, dl_f32)


def kernel(**inputs):
    from concourse.bass_utils import run_bass_kernel_spmd

    nc, in_maps = get_program_and_maps(inputs)
    res = run_bass_kernel_spmd(nc, in_maps, list(range(NCORES)))
    outs = [res.results[k]["out"] for k in range(NCORES)]
    return np.concatenate(outs, axis=0)


# revision 10
# speedup vs baseline: 1.4319x; 1.4319x over previous
"""GAT (3-layer, PyG-style) on 8 Trainium2 NeuronCores via Bass/Tile.

Sharding: edges are partitioned by dst-node range (1250 nodes per core).
Per layer: (A) node-parallel feature projection + attention logits;
AllGather of a node-major fp16 row table [hx_fp16 | al_src_fp32 | pad]
into each core's DRAM (al_dst stays in core-local SBUF); (B)
edge-parallel aggregation: hardware dma_gather of per-edge rows by src
id; al_dst broadcast to edges via a transposed one-hot matmul on the
tensor engine; exp(leaky(logits)) scaling on fp32 logits; segment-sum
via one-hot-mask matmuls accumulated in PSUM (the softmax denominator
rides a second matmul; division is applied per dst node afterwards).
The softmax max-subtraction is skipped: logits are bounded by
construction (|e| < ~8) so exp stays well inside fp16/fp32 range.
"""

import sys

sys.path.insert(0, "/opt/trn_rl_repo")

import numpy as np

N = 10000
E = 160000
NCORES = 8
NS = 1250          # nodes per core
T = 10             # dst tiles per core
TS = 125           # nodes per dst tile
NEG = 0.2          # leaky_relu slope

F_IN = 256
HC = 512
NUM_CLASSES = 40

_cache = {}


def _wrap16(a):
    """[T, CH, 128] per-slot values -> [128, T*CH*8] int16 wrapped layout.

    dma_gather consumes index i from (partition i%16, col i//16), replicated
    across the eight 16-partition groups. Slot (t, j, q) is flat index
    i = j*128 + q within tile t's column block.
    """
    Tn, CHn, _ = a.shape
    b = a.reshape(Tn * CHn * 8, 16).astype(np.int16)
    m = np.ascontiguousarray(b.T)          # [16, T*CH*8]
    return np.tile(m, (8, 1))              # [128, T*CH*8]


def _preprocess(edge_index):
    """Group edges (incl. self-loops) by (core, dst-tile); pad chunks of 128.

    Padding slots keep src=0 (harmless gather) and mask value 127 (>= TS)
    so one-hot mask rows are all-zero and they contribute nothing.
    """
    src = np.concatenate([np.asarray(edge_index[0]), np.arange(N)]).astype(np.int64)
    dst = np.concatenate([np.asarray(edge_index[1]), np.arange(N)]).astype(np.int64)
    gtile = dst // TS                       # global tile id 0..79
    order = np.argsort(gtile, kind="stable")
    src, dst, gtile = src[order], dst[order], gtile[order]
    counts = np.bincount(gtile, minlength=NCORES * T)
    CH = int(np.ceil(counts.max() / 128))
    starts = np.concatenate([[0], np.cumsum(counts)])

    idx16s = np.empty((NCORES, 128, T * CH * 8), np.int16)
    dl16 = np.empty((NCORES, 128, T * CH), np.float16)
    dlT16 = np.empty((NCORES, 128, T * CH * 128), np.float16)
    for k in range(NCORES):
        sa = np.zeros((T, CH, 128), np.int64)
        dl = np.full((T, CH, 128), 127.0, np.float32)
        for t in range(T):
            g = k * T + t
            s0, s1 = starts[g], starts[g + 1]
            m = s1 - s0
            i = np.arange(m)
            js, qs = i // 128, i % 128
            sa[t, js, qs] = src[s0:s1]
            dl[t, js, qs] = (dst[s0:s1] % TS).astype(np.float32)
        idx16s[k] = _wrap16(sa)
        # mask layout: value at (p, t*CH + j) = dl[t, j, p]
        dl16[k] = dl.transpose(2, 0, 1).reshape(128, T * CH).astype(np.float16)
        # maskT layout: value at (p, (t*CH+j)*128 + q) = dl[t, j, q], any p
        dlT16[k] = np.broadcast_to(
            dl.reshape(1, T * CH * 128), (128, T * CH * 128)
        ).astype(np.float16)
    return CH, idx16s, dl16, dlT16


def _feat_major(w, fb):
    """[K, M] -> [128, fb, M] with element (p, b, m) = w[b*128 + p, m]."""
    K, M = w.shape
    assert K == fb * 128
    return np.ascontiguousarray(w.reshape(fb, 128, M).transpose(1, 0, 2))


def _block_diag_a(a_src, a_dst):
    """[H, C] pair -> [H*C, 16] block-diag (cols 0:8 src, 8:16 dst)."""
    h, c = a_src.shape
    blk = np.zeros((h * c, 16), np.float32)
    for i in range(h):
        blk[i * c : (i + 1) * c, i] = a_src[i]
        blk[i * c : (i + 1) * c, 8 + i] = a_dst[i]
    return blk


def _build_program(CH):
    import concourse.bass as bass
    import concourse.mybir as mybir
    import concourse.bacc as bacc
    import concourse.tile as tile

    f32 = mybir.dt.float32
    f16 = mybir.dt.float16
    i16 = mybir.dt.int16
    Alu = mybir.AluOpType
    Act = mybir.ActivationFunctionType

    def bcast_mid(ap, n, axis=1):
        """Insert a stride-0 dim of extent n at `axis` of an AP."""
        newap = [list(d) for d in ap.ap]
        newap.insert(axis, [0, n])
        return bass.AP(ap.tensor, ap.offset, newap)

    def bcast_col(ap2d, n):
        """[P, 1] AP -> [P, n] with stride-0 free dim."""
        return bass.AP(ap2d.tensor, ap2d.offset, [list(ap2d.ap[0]), [0, n]])

    nc = bacc.Bacc(
        "TRN2",
        target_bir_lowering=False,
        debug=False,
        enable_asserts=False,
        num_devices=NCORES,
    )

    icols = T * CH * 8
    NSL = [(0, 512), (512, 512), (1024, 226)]  # 1250 split for matmul N<=512

    # layer specs: (fin_blocks, fout, fout_blocks, H, C, row_f16)
    # row_f16: gathered-table row length in fp16 units (256B-aligned)
    specs = [
        (2, 512, 4, 8, 64, 640),
        (4, 512, 4, 8, 64, 640),
        (4, 40, 1, 1, 40, 128),
    ]

    # ---- external I/O ----
    xt_in = nc.dram_tensor("xt", [128, 2, NS], f32, kind="ExternalInput")
    idxs_in = nc.dram_tensor("idx16s", [128, icols], i16, kind="ExternalInput")
    dstl_in = nc.dram_tensor("dstl16", [128, T * CH], f16, kind="ExternalInput")
    dstlT_in = nc.dram_tensor(
        "dstlT16", [128, T * CH * 128], f16, kind="ExternalInput"
    )
    w_in = [
        nc.dram_tensor("W0", [128, 2, 512], f32, kind="ExternalInput"),
        nc.dram_tensor("W1", [128, 4, 512], f32, kind="ExternalInput"),
        nc.dram_tensor("W2", [128, 4, 40], f32, kind="ExternalInput"),
    ]
    a_in = [
        nc.dram_tensor("A0", [128, 4, 16], f32, kind="ExternalInput"),
        nc.dram_tensor("A1", [128, 4, 16], f32, kind="ExternalInput"),
        nc.dram_tensor("A2", [40, 2], f32, kind="ExternalInput"),
    ]
    b_in = [
        nc.dram_tensor("B0", [128, 512], f32, kind="ExternalInput"),
        nc.dram_tensor("B1", [128, 512], f32, kind="ExternalInput"),
        nc.dram_tensor("B2", [128, 40], f32, kind="ExternalInput"),
    ]
    iota_in = nc.dram_tensor("iota16", [128, 128], f16, kind="ExternalInput")
    iotap_in = nc.dram_tensor("iotaP16", [128, 1], f16, kind="ExternalInput")
    ident_in = nc.dram_tensor("ident", [128, 128], f32, kind="ExternalInput")
    out_ext = nc.dram_tensor("out", [NS, NUM_CLASSES], f32, kind="ExternalOutput")

    # internal DRAM
    agin = [
        nc.dram_tensor(f"agin{l}", [NS, specs[l][5]], f16, kind="Internal")
        for l in range(3)
    ]
    tbl = [
        nc.dram_tensor(
            f"tbl{l}", [N, specs[l][5]], f16, kind="Internal", addr_space="Shared"
        )
        for l in range(3)
    ]

    groups = [list(range(NCORES))]

    with tile.TileContext(nc) as tc:
        with (
            tc.tile_pool(name="const", bufs=1) as cp,
            tc.tile_pool(name="sb", bufs=2) as sb,
            tc.tile_pool(name="persist", bufs=1) as pp,
            tc.tile_pool(name="psA", bufs=1, space="PSUM") as psA,
            tc.tile_pool(name="psT", bufs=2, space="PSUM") as psT,
            tc.tile_pool(name="psO", bufs=2, space="PSUM") as psO,
            tc.tile_pool(name="psD", bufs=1, space="PSUM") as psD,
            tc.tile_pool(name="psL", bufs=2, space="PSUM") as psL,
        ):
            # ---- load constants ----
            def load_const(t_in, shape, dtype=f32):
                t = cp.tile(shape, dtype, tag=t_in.name)
                nc.sync.dma_start(out=t[:], in_=t_in[:])
                return t

            xt0 = pp.tile([128, 2, NS], f32, tag="xt0")
            nc.sync.dma_start(out=xt0[:], in_=xt_in[:])
            idx16s = load_const(idxs_in, [128, icols], i16)
            dstl16 = load_const(dstl_in, [128, T * CH], f16)
            Wt = [
                load_const(w_in[0], [128, 2, 512]),
                load_const(w_in[1], [128, 4, 512]),
                load_const(w_in[2], [128, 4, 40]),
            ]
            At = [
                load_const(a_in[0], [128, 4, 16]),
                load_const(a_in[1], [128, 4, 16]),
                load_const(a_in[2], [40, 2]),
            ]
            Bt = [
                load_const(b_in[0], [128, 512]),
                load_const(b_in[1], [128, 512]),
                load_const(b_in[2], [128, 40]),
            ]
            iota16 = load_const(iota_in, [128, 128], f16)
            iotaP16 = load_const(iotap_in, [128, 1], f16)
            ident = load_const(ident_in, [128, 128])
            aldsb = pp.tile([TS, T, 8], f32, tag="aldsb")

            xt_cur = xt0
            for l, (fbi, fout, fbo, Hh, Cc, row) in enumerate(specs):
                # ============ phase A: hxT = W.T-contract(XT), logits ============
                if l < 2:
                    hxT = pp.tile([128, fbo, NS], f32, tag="hxT")
                else:
                    hxT = pp.tile([40, NS], f32, tag="hxT2")
                alT = pp.tile([16 if l < 2 else 2, NS], f32, tag="alT")

                for fo in range(fbo):
                    mpart = 128 if l < 2 else 40
                    for (n0, nw) in NSL:
                        ps = psA.tile([mpart, nw], f32, tag="big")
                        for fb in range(fbi):
                            nc.tensor.matmul(
                                out=ps[:],
                                lhsT=Wt[l][:, fb, fo * 128 : fo * 128 + mpart],
                                rhs=xt_cur[:, fb, n0 : n0 + nw],
                                start=(fb == 0),
                                stop=(fb == fbi - 1),
                            )
                        if l < 2:
                            nc.vector.tensor_copy(
                                out=hxT[:, fo, n0 : n0 + nw], in_=ps[:]
                            )
                        else:
                            nc.vector.tensor_copy(out=hxT[:, n0 : n0 + nw], in_=ps[:])

                # attention logits alT = A.T @ hxT   (contract over fout)
                napart = 16 if l < 2 else 2
                for (n0, nw) in NSL:
                    ps = psA.tile([napart, nw], f32, tag="big")
                    if l < 2:
                        for fb in range(fbo):
                            nc.tensor.matmul(
                                out=ps[:],
                                lhsT=At[l][:, fb, :napart],
                                rhs=hxT[:, fb, n0 : n0 + nw],
                                start=(fb == 0),
                                stop=(fb == fbo - 1),
                            )
                    else:
                        nc.tensor.matmul(
                            out=ps[:],
                            lhsT=At[l][:40, :2],
                            rhs=hxT[:40, n0 : n0 + nw],
                            start=True,
                            stop=True,
                        )
                    nc.vector.tensor_copy(out=alT[:, n0 : n0 + nw], in_=ps[:])

                # transpose to node-major fp16 rows; al_src as fp32 bytes in-row
                for t in range(T):
                    stg = sb.tile([TS, row], f16, tag="stg")
                    nsl = slice(t * TS, (t + 1) * TS)
                    if l < 2:
                        for fo in range(fbo):
                            pt = psT.tile([TS, 128], f32, tag="tr")
                            nc.tensor.transpose(
                                out=pt[:], in_=hxT[:, fo, nsl], identity=ident[:]
                            )
                            nc.vector.tensor_copy(
                                out=stg[:, fo * 128 : (fo + 1) * 128], in_=pt[:]
                            )
                        pa = psT.tile([TS, 16], f32, tag="tr")
                        nc.tensor.transpose(
                            out=pa[:], in_=alT[:16, nsl], identity=ident[:16, :16]
                        )
                        nc.vector.tensor_copy(
                            out=stg[:, 512:528].bitcast(f32), in_=pa[:, 0:8]
                        )
                        nc.vector.tensor_copy(out=aldsb[:, t, :], in_=pa[:, 8:16])
                        nc.vector.memset(stg[:, 528:row], 0.0)
                    else:
                        pt = psT.tile([TS, 40], f32, tag="tr")
                        nc.tensor.transpose(
                            out=pt[:], in_=hxT[:40, nsl], identity=ident[:40, :40]
                        )
                        nc.vector.tensor_copy(out=stg[:, 0:40], in_=pt[:])
                        pa = psT.tile([TS, 2], f32, tag="tr")
                        nc.tensor.transpose(
                            out=pa[:], in_=alT[:2, nsl], identity=ident[:2, :2]
                        )
                        nc.vector.tensor_copy(
                            out=stg[:, 40:42].bitcast(f32), in_=pa[:, 0:1]
                        )
                        nc.vector.tensor_copy(out=aldsb[:, t, 0:1], in_=pa[:, 1:2])
                        nc.vector.memset(stg[:, 42:row], 0.0)
                    nc.sync.dma_start(out=agin[l][nsl, :], in_=stg[:])

                nc.gpsimd.collective_compute(
                    "AllGather",
                    Alu.bypass,
                    replica_groups=groups,
                    ins=[agin[l][:]],
                    outs=[tbl[l][:]],
                )

                # ============ phase B: edge aggregation ============
                if l < 2:
                    xt_next = pp.tile([128, 4, NS], f32, tag=f"xt{l + 1}")
                hc0 = max(1, (CH + 2) // 3)  # chunks per gather batch
                for t in range(T):
                    # one-hot masks for the whole tile, one DVE op each
                    mk = sb.tile([128, CH, TS], f16, tag="mk")
                    nc.vector.tensor_tensor(
                        out=mk[:],
                        in0=bcast_mid(iota16[:, 0:TS], CH),
                        in1=dstl16[:, t * CH : (t + 1) * CH].to_broadcast(
                            [128, CH, TS]
                        ),
                        op=Alu.is_equal,
                    )
                    dT = sb.tile([128, CH * 128], f16, tag="dT")
                    nc.sync.dma_start(
                        out=dT[:],
                        in_=dstlT_in[:, t * CH * 128 : (t + 1) * CH * 128],
                    )
                    mkT = sb.tile([TS, CH * 128], f32, tag="mkT")
                    nc.vector.tensor_tensor(
                        out=mkT[:],
                        in0=bcast_col(iotaP16[:TS, 0:1], CH * 128),
                        in1=dT[:TS, :],
                        op=Alu.is_equal,
                    )
                    ps_out = psO.tile([TS, fout], f32, tag="out")
                    ps_den = psD.tile([TS, Hh], f32, tag="den")
                    jj = 0
                    while jj < CH:
                        hc = min(hc0, CH - jj)
                        ic0 = (t * CH + jj) * 8
                        G = sb.tile([128, hc0, row], f16, tag="G")
                        nc.gpsimd.dma_gather(
                            out_ap=G[:, :hc, :],
                            in_ap=tbl[l][:],
                            idxs_ap=idx16s[:, ic0 : ic0 + hc * 8],
                            num_idxs=hc * 128,
                            num_idxs_reg=hc * 128,
                            elem_size=row,
                        )
                        # logits: leaky(als[src] + ald[dst]) in fp32, then exp
                        lg = sb.tile([128, hc0, Hh], f32, tag="lg")
                        alo = 512 if l < 2 else 40
                        for j in range(hc):
                            ps_ald = psL.tile([128, Hh], f32, tag="aldpe")
                            nc.tensor.matmul(
                                out=ps_ald[:],
                                lhsT=mkT[:, (jj + j) * 128 : (jj + j + 1) * 128],
                                rhs=aldsb[:, t, :Hh],
                                start=True,
                                stop=True,
                                skip_group_check=True,
                            )
                            nc.vector.tensor_tensor(
                                out=lg[:, j, :],
                                in0=G[:, j, alo : alo + 2 * Hh].bitcast(f32),
                                in1=ps_ald[:],
                                op=Alu.add,
                            )
                        t2 = sb.tile([128, hc0, Hh], f32, tag="t2")
                        nc.vector.tensor_scalar(
                            out=t2[:, :hc, :],
                            in0=lg[:, :hc, :],
                            scalar1=NEG,
                            scalar2=None,
                            op0=Alu.mult,
                        )
                        nc.vector.tensor_tensor(
                            out=lg[:, :hc, :],
                            in0=lg[:, :hc, :],
                            in1=t2[:, :hc, :],
                            op=Alu.max,
                        )
                        ex = sb.tile([128, hc0, Hh], f16, tag="ex")
                        nc.scalar.activation(
                            out=ex[:, :hc, :], in_=lg[:, :hc, :], func=Act.Exp
                        )
                        # scale features by exp(logit) per (edge, head)
                        G2 = sb.tile([128, hc0, Hh, Cc], f16, tag="G2")
                        nc.vector.tensor_tensor(
                            out=G2[:, :hc, :, :],
                            in0=G[:, :hc, 0:fout].rearrange(
                                "p j (h c) -> p j h c", h=Hh
                            ),
                            in1=ex[:, :hc, :].to_broadcast([128, hc, Hh, Cc]),
                            op=Alu.mult,
                        )
                        for j in range(hc):
                            glob_j = jj + j
                            nc.tensor.matmul(
                                out=ps_out[:],
                                lhsT=mk[:, glob_j, :],
                                rhs=G2[:, j],
                                start=(glob_j == 0),
                                stop=(glob_j == CH - 1),
                                skip_group_check=True,
                            )
                            nc.tensor.matmul(
                                out=ps_den[:],
                                lhsT=mk[:, glob_j, :],
                                rhs=ex[:, j],
                                start=(glob_j == 0),
                                stop=(glob_j == CH - 1),
                                skip_group_check=True,
                            )
                        jj += hc
                    # tail: divide by denominator, bias, activation
                    den = sb.tile([TS, Hh], f32, tag="dent")
                    nc.vector.tensor_scalar(
                        out=den[:], in0=ps_den[:], scalar1=1e-16, scalar2=None,
                        op0=Alu.add,
                    )
                    rcp = sb.tile([TS, Hh], f32, tag="rcp")
                    nc.vector.reciprocal(out=rcp[:], in_=den[:])
                    y = sb.tile([TS, fout], f32, tag="y")
                    nc.vector.tensor_tensor(
                        out=y[:].rearrange("p (h c) -> p h c", h=Hh),
                        in0=ps_out[:].rearrange("p (h c) -> p h c", h=Hh),
                        in1=rcp[:].to_broadcast([TS, Hh, Cc]),
                        op=Alu.mult,
                    )
                    nc.vector.tensor_tensor(
                        out=y[:], in0=y[:], in1=Bt[l][:TS, :fout], op=Alu.add
                    )
                    nsl = slice(t * TS, (t + 1) * TS)
                    if l < 2:
                        # elu(y) = max(y,0) + exp(min(y,0)) - 1
                        ymin = sb.tile([TS, fout], f32, tag="ymin")
                        nc.vector.tensor_scalar(
                            out=ymin[:], in0=y[:], scalar1=0.0, scalar2=None,
                            op0=Alu.min,
                        )
                        eneg = sb.tile([TS, fout], f32, tag="eneg")
                        nc.scalar.activation(out=eneg[:], in_=ymin[:], func=Act.Exp)
                        nc.vector.tensor_scalar(
                            out=eneg[:], in0=eneg[:], scalar1=1.0, scalar2=None,
                            op0=Alu.subtract,
                        )
                        x2 = sb.tile([TS, fout], f32, tag="x2")
                        nc.vector.tensor_scalar(
                            out=x2[:], in0=y[:], scalar1=0.0, scalar2=None,
                            op0=Alu.max,
                        )
                        nc.vector.tensor_tensor(
                            out=x2[:], in0=x2[:], in1=eneg[:], op=Alu.add
                        )
                        # transpose into next layer's feature-major XT
                        for fo in range(4):
                            pt = psT.tile([128, TS], f32, tag="tr")
                            nc.tensor.transpose(
                                out=pt[:],
                                in_=x2[:, fo * 128 : (fo + 1) * 128],
                                identity=ident[:TS, :TS],
                            )
                            nc.vector.tensor_copy(out=xt_next[:, fo, nsl], in_=pt[:])
                    else:
                        nc.sync.dma_start(out=out_ext[nsl, :], in_=y[:, :NUM_CLASSES])
                if l < 2:
                    xt_cur = xt_next

    nc.compile()
    return nc


def _make_in_maps(inputs, CH, idx16s, dl16, dlT16):
    x = np.asarray(inputs["x"], np.float32)
    iota16 = np.tile(np.arange(128, dtype=np.float16), (128, 1))
    ident = np.eye(128, dtype=np.float32)
    common = {
        "W0": _feat_major(np.asarray(inputs["W0"], np.float32), 2),
        "W1": _feat_major(np.asarray(inputs["W1"], np.float32), 4),
        "W2": _feat_major(np.asarray(inputs["W2"], np.float32), 4),
        "A0": _feat_major(
            _block_diag_a(np.asarray(inputs["a_src0"]), np.asarray(inputs["a_dst0"])), 4
        ),
        "A1": _feat_major(
            _block_diag_a(np.asarray(inputs["a_src1"]), np.asarray(inputs["a_dst1"])), 4
        ),
        "A2": np.ascontiguousarray(
            np.stack(
                [
                    np.asarray(inputs["a_src2"], np.float32)[0],
                    np.asarray(inputs["a_dst2"], np.float32)[0],
                ],
                axis=1,
            )
        ),
        "B0": np.tile(np.asarray(inputs["b0"], np.float32), (128, 1)),
        "B1": np.tile(np.asarray(inputs["b1"], np.float32), (128, 1)),
        "B2": np.tile(np.asarray(inputs["b2"], np.float32), (128, 1)),
        "iota16": iota16,
        "iotaP16": np.arange(128, dtype=np.float16).reshape(128, 1),
        "ident": ident,
    }
    in_maps = []
    for k in range(NCORES):
        xs = x[k * NS : (k + 1) * NS]  # [NS, 256]
        xt = np.ascontiguousarray(xs.T.reshape(2, 128, NS).transpose(1, 0, 2))
        in_maps.append(
            dict(
                common,
                xt=xt,
                idx16s=idx16s[k],
                dstl16=dl16[k],
                dstlT16=np.ascontiguousarray(dlT16[k]),
            )
        )
    return in_maps


def get_program_and_maps(inputs):
    CH, idx16s, dl16, dlT16 = _preprocess(np.asarray(inputs["edge_index"]))
    if CH not in _cache:
        _cache[CH] = _build_program(CH)
    nc = _cache[CH]
    return nc, _make_in_maps(inputs, CH, idx16s, dl16, dlT16)


def kernel(**inputs):
    from concourse.bass_utils import run_bass_kernel_spmd

    nc, in_maps = get_program_and_maps(inputs)
    res = run_bass_kernel_spmd(nc, in_maps, list(range(NCORES)))
    outs = [res.results[k]["out"] for k in range(NCORES)]
    return np.concatenate(outs, axis=0)


# revision 15
# speedup vs baseline: 1.8392x; 1.2845x over previous
"""GAT (3-layer, PyG-style) on 8 Trainium2 NeuronCores via Bass/Tile.

Sharding: edges are partitioned by dst-node range (1250 nodes per core).
Per layer: (A) node-parallel feature projection + attention logits;
AllGather of a node-major fp16 row table [hx_fp16 | al_src_fp32 | pad]
into each core's DRAM (al_dst stays in core-local SBUF); (B)
edge-parallel aggregation: hardware dma_gather of per-edge rows by src
id; al_dst broadcast to edges via a transposed one-hot matmul on the
tensor engine; exp(leaky(logits)) scaling on fp32 logits; segment-sum
via one-hot-mask matmuls accumulated in PSUM (the softmax denominator
rides a second matmul; division is applied per dst node afterwards).
The softmax max-subtraction is skipped: logits are bounded by
construction (|e| < ~8) so exp stays well inside fp16/fp32 range.
"""

import sys

sys.path.insert(0, "/opt/trn_rl_repo")

import numpy as np

N = 10000
E = 160000
NCORES = 8
NS = 1250          # nodes per core
T = 10             # dst tiles per core
TS = 125           # nodes per dst tile
NEG = 0.2          # leaky_relu slope

F_IN = 256
HC = 512
NUM_CLASSES = 40

_cache = {}


def _wrap16(a):
    """[T, CH, 128] per-slot values -> [128, T*CH*8] int16 wrapped layout.

    dma_gather consumes index i from (partition i%16, col i//16), replicated
    across the eight 16-partition groups. Slot (t, j, q) is flat index
    i = j*128 + q within tile t's column block.
    """
    Tn, CHn, _ = a.shape
    b = a.reshape(Tn * CHn * 8, 16).astype(np.int16)
    m = np.ascontiguousarray(b.T)          # [16, T*CH*8]
    return np.tile(m, (8, 1))              # [128, T*CH*8]


def _preprocess(edge_index):
    """Group edges (incl. self-loops) by (core, dst-tile); pad chunks of 128.

    Padding slots keep src=0 (harmless gather) and mask value 127 (>= TS)
    so one-hot mask rows are all-zero and they contribute nothing.
    """
    src = np.concatenate([np.asarray(edge_index[0]), np.arange(N)]).astype(np.int64)
    dst = np.concatenate([np.asarray(edge_index[1]), np.arange(N)]).astype(np.int64)
    gtile = dst // TS                       # global tile id 0..79
    order = np.argsort(gtile, kind="stable")
    src, dst, gtile = src[order], dst[order], gtile[order]
    counts = np.bincount(gtile, minlength=NCORES * T)
    CH = int(np.ceil(counts.max() / 128))
    starts = np.concatenate([[0], np.cumsum(counts)])

    idx16s = np.empty((NCORES, 128, T * CH * 8), np.int16)
    dl16 = np.empty((NCORES, 128, T * CH), np.float16)
    dlT16 = np.empty((NCORES, 128, T * CH * 128), np.float16)
    for k in range(NCORES):
        sa = np.zeros((T, CH, 128), np.int64)
        dl = np.full((T, CH, 128), 127.0, np.float32)
        for t in range(T):
            g = k * T + t
            s0, s1 = starts[g], starts[g + 1]
            m = s1 - s0
            i = np.arange(m)
            js, qs = i // 128, i % 128
            sa[t, js, qs] = src[s0:s1]
            dl[t, js, qs] = (dst[s0:s1] % TS).astype(np.float32)
        idx16s[k] = _wrap16(sa)
        # mask layout: value at (p, t*CH + j) = dl[t, j, p]
        dl16[k] = dl.transpose(2, 0, 1).reshape(128, T * CH).astype(np.float16)
        # maskT layout: value at (p, (t*CH+j)*128 + q) = dl[t, j, q], any p
        dlT16[k] = np.broadcast_to(
            dl.reshape(1, T * CH * 128), (128, T * CH * 128)
        ).astype(np.float16)
    return CH, idx16s, dl16, dlT16


def _feat_major(w, fb):
    """[K, M] -> [128, fb, M] with element (p, b, m) = w[b*128 + p, m]."""
    K, M = w.shape
    assert K == fb * 128
    return np.ascontiguousarray(w.reshape(fb, 128, M).transpose(1, 0, 2))


def _block_diag_a(a_src, a_dst):
    """[H, C] pair -> [H*C, 16] block-diag (cols 0:8 src, 8:16 dst)."""
    h, c = a_src.shape
    blk = np.zeros((h * c, 16), np.float32)
    for i in range(h):
        blk[i * c : (i + 1) * c, i] = a_src[i]
        blk[i * c : (i + 1) * c, 8 + i] = a_dst[i]
    return blk


def _build_program(CH):
    import concourse.bass as bass
    import concourse.mybir as mybir
    import concourse.bacc as bacc
    import concourse.tile as tile

    f32 = mybir.dt.float32
    f16 = mybir.dt.float16
    i16 = mybir.dt.int16
    Alu = mybir.AluOpType
    Act = mybir.ActivationFunctionType

    def bcast_mid(ap, n, axis=1):
        """Insert a stride-0 dim of extent n at `axis` of an AP."""
        newap = [list(d) for d in ap.ap]
        newap.insert(axis, [0, n])
        return bass.AP(ap.tensor, ap.offset, newap)

    def bcast_col(ap2d, n):
        """[P, 1] AP -> [P, n] with stride-0 free dim."""
        return bass.AP(ap2d.tensor, ap2d.offset, [list(ap2d.ap[0]), [0, n]])

    nc = bacc.Bacc(
        "TRN2",
        target_bir_lowering=False,
        debug=False,
        enable_asserts=False,
        num_devices=NCORES,
    )

    icols = T * CH * 8
    NSL = [(0, 512), (512, 512), (1024, 226)]  # 1250 split for matmul N<=512

    # layer specs: (fin_blocks, fout, fout_blocks, H, C, row_f16)
    # row_f16: gathered-table row length in fp16 units (256B-aligned)
    specs = [
        (2, 512, 4, 8, 64, 640),
        (4, 512, 4, 8, 64, 640),
        (4, 40, 1, 1, 40, 128),
    ]

    # ---- external I/O ----
    xt_in = nc.dram_tensor("xt", [128, 2, NS], f32, kind="ExternalInput")
    idxs_in = nc.dram_tensor("idx16s", [128, icols], i16, kind="ExternalInput")
    dstl_in = nc.dram_tensor("dstl16", [128, T * CH], f16, kind="ExternalInput")
    dstlT_in = nc.dram_tensor(
        "dstlT16", [128, T * CH * 128], f16, kind="ExternalInput"
    )
    w_in = [
        nc.dram_tensor("W0", [128, 2, 512], f32, kind="ExternalInput"),
        nc.dram_tensor("W1", [128, 4, 512], f32, kind="ExternalInput"),
        nc.dram_tensor("W2", [128, 4, 40], f32, kind="ExternalInput"),
    ]
    a_in = [
        nc.dram_tensor("A0", [128, 4, 16], f32, kind="ExternalInput"),
        nc.dram_tensor("A1", [128, 4, 16], f32, kind="ExternalInput"),
        nc.dram_tensor("A2", [40, 2], f32, kind="ExternalInput"),
    ]
    b_in = [
        nc.dram_tensor("B0", [128, 512], f32, kind="ExternalInput"),
        nc.dram_tensor("B1", [128, 512], f32, kind="ExternalInput"),
        nc.dram_tensor("B2", [128, 40], f32, kind="ExternalInput"),
    ]
    iota_in = nc.dram_tensor("iota16", [128, 128], f16, kind="ExternalInput")
    iotap_in = nc.dram_tensor("iotaP16", [128, 1], f16, kind="ExternalInput")
    ident_in = nc.dram_tensor("ident", [128, 128], f32, kind="ExternalInput")
    out_ext = nc.dram_tensor("out", [NS, NUM_CLASSES], f32, kind="ExternalOutput")

    # internal DRAM
    agin = [
        nc.dram_tensor(f"agin{l}", [NS, specs[l][5]], f16, kind="Internal")
        for l in range(3)
    ]
    tbl = [
        nc.dram_tensor(
            f"tbl{l}", [N, specs[l][5]], f16, kind="Internal", addr_space="Shared"
        )
        for l in range(3)
    ]

    groups = [list(range(NCORES))]

    with tile.TileContext(nc) as tc:
        with (
            tc.tile_pool(name="const", bufs=1) as cp,
            tc.tile_pool(name="sb", bufs=2) as sb,
            tc.tile_pool(name="sb3", bufs=3) as sb3,
            tc.tile_pool(name="persist", bufs=1) as pp,
            tc.tile_pool(name="psA", bufs=1, space="PSUM") as psA,
            tc.tile_pool(name="psT", bufs=2, space="PSUM") as psT,
            tc.tile_pool(name="psO", bufs=2, space="PSUM") as psO,
            tc.tile_pool(name="psD", bufs=1, space="PSUM") as psD,
            tc.tile_pool(name="psL", bufs=2, space="PSUM") as psL,
        ):
            # ---- load constants ----
            def load_const(t_in, shape, dtype=f32):
                t = cp.tile(shape, dtype, tag=t_in.name)
                nc.sync.dma_start(out=t[:], in_=t_in[:])
                return t

            xt0 = pp.tile([128, 2, NS], f32, tag="xt0")
            nc.sync.dma_start(out=xt0[:], in_=xt_in[:])
            idx16s = load_const(idxs_in, [128, icols], i16)
            dstl16 = load_const(dstl_in, [128, T * CH], f16)
            Wt = [
                load_const(w_in[0], [128, 2, 512]),
                load_const(w_in[1], [128, 4, 512]),
                load_const(w_in[2], [128, 4, 40]),
            ]
            At = [
                load_const(a_in[0], [128, 4, 16]),
                load_const(a_in[1], [128, 4, 16]),
                load_const(a_in[2], [40, 2]),
            ]
            Bt = [
                load_const(b_in[0], [128, 512]),
                load_const(b_in[1], [128, 512]),
                load_const(b_in[2], [128, 40]),
            ]
            iota16 = load_const(iota_in, [128, 128], f16)
            iotaP16 = load_const(iotap_in, [128, 1], f16)
            ident = load_const(ident_in, [128, 128])
            aldsb = pp.tile([TS, T, 8], f16, tag="aldsb")

            xt_cur = xt0
            for l, (fbi, fout, fbo, Hh, Cc, row) in enumerate(specs):
                # ============ phase A: hxT = W.T-contract(XT), logits ============
                if l < 2:
                    hxT = pp.tile([128, fbo, NS], f32, tag="hxT")
                else:
                    hxT = pp.tile([40, NS], f32, tag="hxT2")
                alT = pp.tile([16 if l < 2 else 2, NS], f32, tag="alT")

                for fo in range(fbo):
                    mpart = 128 if l < 2 else 40
                    for (n0, nw) in NSL:
                        ps = psA.tile([mpart, nw], f32, tag="big")
                        for fb in range(fbi):
                            nc.tensor.matmul(
                                out=ps[:],
                                lhsT=Wt[l][:, fb, fo * 128 : fo * 128 + mpart],
                                rhs=xt_cur[:, fb, n0 : n0 + nw],
                                start=(fb == 0),
                                stop=(fb == fbi - 1),
                            )
                        if l < 2:
                            nc.vector.tensor_copy(
                                out=hxT[:, fo, n0 : n0 + nw], in_=ps[:]
                            )
                        else:
                            nc.vector.tensor_copy(out=hxT[:, n0 : n0 + nw], in_=ps[:])

                # attention logits alT = A.T @ hxT   (contract over fout)
                napart = 16 if l < 2 else 2
                for (n0, nw) in NSL:
                    ps = psA.tile([napart, nw], f32, tag="big")
                    if l < 2:
                        for fb in range(fbo):
                            nc.tensor.matmul(
                                out=ps[:],
                                lhsT=At[l][:, fb, :napart],
                                rhs=hxT[:, fb, n0 : n0 + nw],
                                start=(fb == 0),
                                stop=(fb == fbo - 1),
                            )
                    else:
                        nc.tensor.matmul(
                            out=ps[:],
                            lhsT=At[l][:40, :2],
                            rhs=hxT[:40, n0 : n0 + nw],
                            start=True,
                            stop=True,
                        )
                    nc.vector.tensor_copy(out=alT[:, n0 : n0 + nw], in_=ps[:])

                # transpose to node-major fp16 rows; al_src as fp32 bytes in-row
                for t in range(T):
                    stg = sb.tile([TS, row], f16, tag="stg")
                    nsl = slice(t * TS, (t + 1) * TS)
                    if l < 2:
                        for fo in range(fbo):
                            pt = psT.tile([TS, 128], f32, tag="tr")
                            nc.tensor.transpose(
                                out=pt[:], in_=hxT[:, fo, nsl], identity=ident[:]
                            )
                            nc.vector.tensor_copy(
                                out=stg[:, fo * 128 : (fo + 1) * 128], in_=pt[:]
                            )
                        pa = psT.tile([TS, 16], f32, tag="tr")
                        nc.tensor.transpose(
                            out=pa[:], in_=alT[:16, nsl], identity=ident[:16, :16]
                        )
                        nc.vector.tensor_copy(
                            out=stg[:, 512:528].bitcast(f32), in_=pa[:, 0:8]
                        )
                        nc.vector.tensor_copy(out=aldsb[:, t, :], in_=pa[:, 8:16])
                        nc.vector.memset(stg[:, 528:row], 0.0)
                    else:
                        pt = psT.tile([TS, 40], f32, tag="tr")
                        nc.tensor.transpose(
                            out=pt[:], in_=hxT[:40, nsl], identity=ident[:40, :40]
                        )
                        nc.vector.tensor_copy(out=stg[:, 0:40], in_=pt[:])
                        pa = psT.tile([TS, 2], f32, tag="tr")
                        nc.tensor.transpose(
                            out=pa[:], in_=alT[:2, nsl], identity=ident[:2, :2]
                        )
                        nc.vector.tensor_copy(
                            out=stg[:, 40:42].bitcast(f32), in_=pa[:, 0:1]
                        )
                        nc.vector.tensor_copy(out=aldsb[:, t, 0:1], in_=pa[:, 1:2])
                        nc.vector.memset(stg[:, 42:row], 0.0)
                    nc.sync.dma_start(out=agin[l][nsl, :], in_=stg[:])

                nc.gpsimd.collective_compute(
                    "AllGather",
                    Alu.bypass,
                    replica_groups=groups,
                    ins=[agin[l][:]],
                    outs=[tbl[l][:]],
                )

                # ============ phase B: edge aggregation ============
                if l < 2:
                    xt_next = pp.tile([128, 4, NS], f32, tag=f"xt{l + 1}")
                hc0 = max(1, (CH + 2) // 3)  # chunks per gather batch
                for t in range(T):
                    # one-hot masks for the whole tile, one DVE op each
                    mk = sb.tile([128, CH, TS], f16, tag="mk")
                    nc.vector.tensor_tensor(
                        out=mk[:],
                        in0=bcast_mid(iota16[:, 0:TS], CH),
                        in1=dstl16[:, t * CH : (t + 1) * CH].to_broadcast(
                            [128, CH, TS]
                        ),
                        op=Alu.is_equal,
                    )
                    dT = sb.tile([128, CH * 128], f16, tag="dT")
                    nc.sync.dma_start(
                        out=dT[:],
                        in_=dstlT_in[:, t * CH * 128 : (t + 1) * CH * 128],
                    )
                    mkT = sb.tile([TS, CH * 128], f16, tag="mkT")
                    nc.vector.tensor_tensor(
                        out=mkT[:],
                        in0=bcast_col(iotaP16[:TS, 0:1], CH * 128),
                        in1=dT[:TS, :],
                        op=Alu.is_equal,
                    )
                    ps_out = psO.tile([TS, fout], f32, tag="out")
                    ps_den = psD.tile([TS, Hh], f32, tag="den")
                    jj = 0
                    while jj < CH:
                        hc = min(hc0, CH - jj)
                        ic0 = (t * CH + jj) * 8
                        G = sb3.tile([128, hc0, row], f16, tag="G")
                        nc.gpsimd.dma_gather(
                            out_ap=G[:, :hc, :],
                            in_ap=tbl[l][:],
                            idxs_ap=idx16s[:, ic0 : ic0 + hc * 8],
                            num_idxs=hc * 128,
                            num_idxs_reg=hc * 128,
                            elem_size=row,
                        )
                        # logits: leaky(als[src] + ald[dst]) in fp32, then exp
                        lg = sb.tile([128, hc0, Hh], f32, tag="lg")
                        alo = 512 if l < 2 else 40
                        ps_ald = psL.tile([128, hc0 * Hh], f32, tag="aldpe")
                        for j in range(hc):
                            nc.tensor.matmul(
                                out=ps_ald[:, j * Hh : (j + 1) * Hh],
                                lhsT=mkT[:, (jj + j) * 128 : (jj + j + 1) * 128],
                                rhs=aldsb[:, t, :Hh],
                                start=True,
                                stop=True,
                                skip_group_check=True,
                            )
                        nc.vector.tensor_tensor(
                            out=lg[:, :hc, :],
                            in0=G[:, :hc, alo : alo + 2 * Hh].bitcast(f32),
                            in1=ps_ald[:].rearrange("p (j h) -> p j h", h=Hh)[
                                :, :hc, :
                            ],
                            op=Alu.add,
                        )
                        t2 = sb.tile([128, hc0, Hh], f32, tag="t2")
                        nc.vector.tensor_scalar(
                            out=t2[:, :hc, :],
                            in0=lg[:, :hc, :],
                            scalar1=NEG,
                            scalar2=None,
                            op0=Alu.mult,
                        )
                        nc.vector.tensor_tensor(
                            out=lg[:, :hc, :],
                            in0=lg[:, :hc, :],
                            in1=t2[:, :hc, :],
                            op=Alu.max,
                        )
                        ex = sb.tile([128, hc0, Hh], f16, tag="ex")
                        nc.scalar.activation(
                            out=ex[:, :hc, :], in_=lg[:, :hc, :], func=Act.Exp
                        )
                        # scale features by exp(logit) per (edge, head)
                        G2 = sb3.tile([128, hc0, Hh, Cc], f16, tag="G2")
                        nc.vector.tensor_tensor(
                            out=G2[:, :hc, :, :],
                            in0=G[:, :hc, 0:fout].rearrange(
                                "p j (h c) -> p j h c", h=Hh
                            ),
                            in1=ex[:, :hc, :].to_broadcast([128, hc, Hh, Cc]),
                            op=Alu.mult,
                        )
                        for j in range(hc):
                            glob_j = jj + j
                            nc.tensor.matmul(
                                out=ps_out[:],
                                lhsT=mk[:, glob_j, :],
                                rhs=G2[:, j],
                                start=(glob_j == 0),
                                stop=(glob_j == CH - 1),
                                skip_group_check=True,
                            )
                            nc.tensor.matmul(
                                out=ps_den[:],
                                lhsT=mk[:, glob_j, :],
                                rhs=ex[:, j],
                                start=(glob_j == 0),
                                stop=(glob_j == CH - 1),
                                skip_group_check=True,
                            )
                        jj += hc
                    # tail: divide by denominator, bias, activation
                    den = sb.tile([TS, Hh], f32, tag="dent")
                    nc.vector.tensor_scalar(
                        out=den[:], in0=ps_den[:], scalar1=1e-16, scalar2=None,
                        op0=Alu.add,
                    )
                    rcp = sb.tile([TS, Hh], f32, tag="rcp")
                    nc.vector.reciprocal(out=rcp[:], in_=den[:])
                    y = sb.tile([TS, fout], f32, tag="y")
                    nc.vector.tensor_tensor(
                        out=y[:].rearrange("p (h c) -> p h c", h=Hh),
                        in0=ps_out[:].rearrange("p (h c) -> p h c", h=Hh),
                        in1=rcp[:].to_broadcast([TS, Hh, Cc]),
                        op=Alu.mult,
                    )
                    nc.vector.tensor_tensor(
                        out=y[:], in0=y[:], in1=Bt[l][:TS, :fout], op=Alu.add
                    )
                    nsl = slice(t * TS, (t + 1) * TS)
                    if l < 2:
                        # elu(y) = max(y,0) + exp(min(y,0)) - 1
                        ymin = sb.tile([TS, fout], f32, tag="ymin")
                        nc.vector.tensor_scalar(
                            out=ymin[:], in0=y[:], scalar1=0.0, scalar2=None,
                            op0=Alu.min,
                        )
                        eneg = sb.tile([TS, fout], f32, tag="eneg")
                        nc.scalar.activation(out=eneg[:], in_=ymin[:], func=Act.Exp)
                        nc.vector.tensor_scalar(
                            out=eneg[:], in0=eneg[:], scalar1=1.0, scalar2=None,
                            op0=Alu.subtract,
                        )
                        x2 = sb.tile([TS, fout], f32, tag="x2")
                        nc.vector.tensor_scalar(
                            out=x2[:], in0=y[:], scalar1=0.0, scalar2=None,
                            op0=Alu.max,
                        )
                        nc.vector.tensor_tensor(
                            out=x2[:], in0=x2[:], in1=eneg[:], op=Alu.add
                        )
                        # transpose into next layer's feature-major XT
                        for fo in range(4):
                            pt = psT.tile([128, TS], f32, tag="tr")
                            nc.tensor.transpose(
                                out=pt[:],
                                in_=x2[:, fo * 128 : (fo + 1) * 128],
                                identity=ident[:TS, :TS],
                            )
                            nc.vector.tensor_copy(out=xt_next[:, fo, nsl], in_=pt[:])
                    else:
                        nc.sync.dma_start(out=out_ext[nsl, :], in_=y[:, :NUM_CLASSES])
                if l < 2:
                    xt_cur = xt_next

    nc.compile()
    return nc


def _make_in_maps(inputs, CH, idx16s, dl16, dlT16):
    x = np.asarray(inputs["x"], np.float32)
    iota16 = np.tile(np.arange(128, dtype=np.float16), (128, 1))
    ident = np.eye(128, dtype=np.float32)
    common = {
        "W0": _feat_major(np.asarray(inputs["W0"], np.float32), 2),
        "W1": _feat_major(np.asarray(inputs["W1"], np.float32), 4),
        "W2": _feat_major(np.asarray(inputs["W2"], np.float32), 4),
        "A0": _feat_major(
            _block_diag_a(np.asarray(inputs["a_src0"]), np.asarray(inputs["a_dst0"])), 4
        ),
        "A1": _feat_major(
            _block_diag_a(np.asarray(inputs["a_src1"]), np.asarray(inputs["a_dst1"])), 4
        ),
        "A2": np.ascontiguousarray(
            np.stack(
                [
                    np.asarray(inputs["a_src2"], np.float32)[0],
                    np.asarray(inputs["a_dst2"], np.float32)[0],
                ],
                axis=1,
            )
        ),
        "B0": np.tile(np.asarray(inputs["b0"], np.float32), (128, 1)),
        "B1": np.tile(np.asarray(inputs["b1"], np.float32), (128, 1)),
        "B2": np.tile(np.asarray(inputs["b2"], np.float32), (128, 1)),
        "iota16": iota16,
        "iotaP16": np.arange(128, dtype=np.float16).reshape(128, 1),
        "ident": ident,
    }
    in_maps = []
    for k in range(NCORES):
        xs = x[k * NS : (k + 1) * NS]  # [NS, 256]
        xt = np.ascontiguousarray(xs.T.reshape(2, 128, NS).transpose(1, 0, 2))
        in_maps.append(
            dict(
                common,
                xt=xt,
                idx16s=idx16s[k],
                dstl16=dl16[k],
                dstlT16=np.ascontiguousarray(dlT16[k]),
            )
        )
    return in_maps


def get_program_and_maps(inputs):
    CH, idx16s, dl16, dlT16 = _preprocess(np.asarray(inputs["edge_index"]))
    if CH not in _cache:
        _cache[CH] = _build_program(CH)
    nc = _cache[CH]
    return nc, _make_in_maps(inputs, CH, idx16s, dl16, dlT16)


def kernel(**inputs):
    from concourse.bass_utils import run_bass_kernel_spmd

    nc, in_maps = get_program_and_maps(inputs)
    res = run_bass_kernel_spmd(nc, in_maps, list(range(NCORES)))
    outs = [res.results[k]["out"] for k in range(NCORES)]
    return np.concatenate(outs, axis=0)


# revision 19
# speedup vs baseline: 1.8834x; 1.0240x over previous
"""GAT (3-layer, PyG-style) on 8 Trainium2 NeuronCores via Bass/Tile.

Sharding: edges are partitioned by dst-node range (1250 nodes per core).
Per layer: (A) node-parallel feature projection + attention logits;
AllGather of a node-major fp16 row table [hx_fp16 | al_src_fp32 | pad]
into each core's DRAM (al_dst stays in core-local SBUF); (B)
edge-parallel aggregation: hardware dma_gather of per-edge rows by src
id; al_dst broadcast to edges via a transposed one-hot matmul on the
tensor engine; exp(leaky(logits)) scaling on fp32 logits; segment-sum
via one-hot-mask matmuls accumulated in PSUM (the softmax denominator
rides a second matmul; division is applied per dst node afterwards).
The softmax max-subtraction is skipped: logits are bounded by
construction (|e| < ~8) so exp stays well inside fp16/fp32 range.
"""

import sys

sys.path.insert(0, "/opt/trn_rl_repo")

import numpy as np

N = 10000
E = 160000
NCORES = 8
NS = 1250          # nodes per core
T = 10             # dst tiles per core
TS = 125           # nodes per dst tile
NEG = 0.2          # leaky_relu slope

F_IN = 256
HC = 512
NUM_CLASSES = 40

_cache = {}


def _wrap16(a):
    """[T, CH, 128] per-slot values -> [128, T*CH*8] int16 wrapped layout.

    dma_gather consumes index i from (partition i%16, col i//16), replicated
    across the eight 16-partition groups. Slot (t, j, q) is flat index
    i = j*128 + q within tile t's column block.
    """
    Tn, CHn, _ = a.shape
    b = a.reshape(Tn * CHn * 8, 16).astype(np.int16)
    m = np.ascontiguousarray(b.T)          # [16, T*CH*8]
    return np.tile(m, (8, 1))              # [128, T*CH*8]


def _preprocess(edge_index):
    """Group edges (incl. self-loops) by (core, dst-tile); pad chunks of 128.

    Padding slots keep src=0 (harmless gather) and mask value 127 (>= TS)
    so one-hot mask rows are all-zero and they contribute nothing.
    """
    src = np.concatenate([np.asarray(edge_index[0]), np.arange(N)]).astype(np.int64)
    dst = np.concatenate([np.asarray(edge_index[1]), np.arange(N)]).astype(np.int64)
    gtile = dst // TS                       # global tile id 0..79
    order = np.argsort(gtile, kind="stable")
    src, dst, gtile = src[order], dst[order], gtile[order]
    counts = np.bincount(gtile, minlength=NCORES * T)
    CH = int(np.ceil(counts.max() / 128))
    starts = np.concatenate([[0], np.cumsum(counts)])

    idx16s = np.empty((NCORES, 128, T * CH * 8), np.int16)
    dl16 = np.empty((NCORES, 128, T * CH), np.float16)
    dlT16 = np.empty((NCORES, 128, T * CH * 128), np.float16)
    for k in range(NCORES):
        sa = np.zeros((T, CH, 128), np.int64)
        dl = np.full((T, CH, 128), 127.0, np.float32)
        for t in range(T):
            g = k * T + t
            s0, s1 = starts[g], starts[g + 1]
            m = s1 - s0
            i = np.arange(m)
            js, qs = i // 128, i % 128
            sa[t, js, qs] = src[s0:s1]
            dl[t, js, qs] = (dst[s0:s1] % TS).astype(np.float32)
        idx16s[k] = _wrap16(sa)
        # mask layout: value at (p, t*CH + j) = dl[t, j, p]
        dl16[k] = dl.transpose(2, 0, 1).reshape(128, T * CH).astype(np.float16)
        # maskT layout: value at (p, (t*CH+j)*128 + q) = dl[t, j, q], any p
        dlT16[k] = np.broadcast_to(
            dl.reshape(1, T * CH * 128), (128, T * CH * 128)
        ).astype(np.float16)
    return CH, idx16s, dl16, dlT16


def _feat_major(w, fb):
    """[K, M] -> [128, fb, M] with element (p, b, m) = w[b*128 + p, m]."""
    K, M = w.shape
    assert K == fb * 128
    return np.ascontiguousarray(w.reshape(fb, 128, M).transpose(1, 0, 2))


def _block_diag_a(a_src, a_dst):
    """[H, C] pair -> [H*C, 16] block-diag (cols 0:8 src, 8:16 dst)."""
    h, c = a_src.shape
    blk = np.zeros((h * c, 16), np.float32)
    for i in range(h):
        blk[i * c : (i + 1) * c, i] = a_src[i]
        blk[i * c : (i + 1) * c, 8 + i] = a_dst[i]
    return blk


def _build_program(CH):
    import concourse.bass as bass
    import concourse.mybir as mybir
    import concourse.bacc as bacc
    import concourse.tile as tile

    f32 = mybir.dt.float32
    f16 = mybir.dt.float16
    i16 = mybir.dt.int16
    Alu = mybir.AluOpType
    Act = mybir.ActivationFunctionType

    def bcast_mid(ap, n, axis=1):
        """Insert a stride-0 dim of extent n at `axis` of an AP."""
        newap = [list(d) for d in ap.ap]
        newap.insert(axis, [0, n])
        return bass.AP(ap.tensor, ap.offset, newap)

    def bcast_col(ap2d, n):
        """[P, 1] AP -> [P, n] with stride-0 free dim."""
        return bass.AP(ap2d.tensor, ap2d.offset, [list(ap2d.ap[0]), [0, n]])

    nc = bacc.Bacc(
        "TRN2",
        target_bir_lowering=False,
        debug=False,
        enable_asserts=False,
        num_devices=NCORES,
    )

    icols = T * CH * 8
    NSL = [(0, 512), (512, 512), (1024, 226)]  # 1250 split for matmul N<=512

    # layer specs: (fin_blocks, fout, fout_blocks, H, C, row_f16)
    # row_f16: gathered-table row length in fp16 units (256B-aligned)
    specs = [
        (2, 512, 4, 8, 64, 640),
        (4, 512, 4, 8, 64, 640),
        (4, 40, 1, 1, 40, 128),
    ]

    # ---- external I/O ----
    xt_in = nc.dram_tensor("xt", [128, 2, NS], f32, kind="ExternalInput")
    idxs_in = nc.dram_tensor("idx16s", [128, icols], i16, kind="ExternalInput")
    dstl_in = nc.dram_tensor("dstl16", [128, T * CH], f16, kind="ExternalInput")
    dstlT_in = nc.dram_tensor(
        "dstlT16", [128, T * CH * 128], f16, kind="ExternalInput"
    )
    w_in = [
        nc.dram_tensor("W0", [128, 2, 512], f32, kind="ExternalInput"),
        nc.dram_tensor("W1", [128, 4, 512], f32, kind="ExternalInput"),
        nc.dram_tensor("W2", [128, 4, 40], f32, kind="ExternalInput"),
    ]
    a_in = [
        nc.dram_tensor("A0", [128, 4, 16], f32, kind="ExternalInput"),
        nc.dram_tensor("A1", [128, 4, 16], f32, kind="ExternalInput"),
        nc.dram_tensor("A2", [40, 2], f32, kind="ExternalInput"),
    ]
    b_in = [
        nc.dram_tensor("B0", [128, 512], f32, kind="ExternalInput"),
        nc.dram_tensor("B1", [128, 512], f32, kind="ExternalInput"),
        nc.dram_tensor("B2", [128, 40], f32, kind="ExternalInput"),
    ]
    iota_in = nc.dram_tensor("iota16", [128, 128], f16, kind="ExternalInput")
    iotap_in = nc.dram_tensor("iotaP16", [128, 1], f16, kind="ExternalInput")
    ident_in = nc.dram_tensor("ident", [128, 128], f32, kind="ExternalInput")
    out_ext = nc.dram_tensor("out", [NS, NUM_CLASSES], f32, kind="ExternalOutput")

    # internal DRAM
    agin = [
        nc.dram_tensor(f"agin{l}", [NS, specs[l][5]], f16, kind="Internal")
        for l in range(3)
    ]
    tbl = [
        nc.dram_tensor(
            f"tbl{l}", [N, specs[l][5]], f16, kind="Internal", addr_space="Shared"
        )
        for l in range(3)
    ]

    groups = [list(range(NCORES))]

    with tile.TileContext(nc) as tc:
        with (
            tc.tile_pool(name="const", bufs=1) as cp,
            tc.tile_pool(name="sb", bufs=2) as sb,
            tc.tile_pool(name="sb3", bufs=3) as sb3,
            tc.tile_pool(name="persist", bufs=1) as pp,
            tc.tile_pool(name="psA", bufs=1, space="PSUM") as psA,
            tc.tile_pool(name="psT", bufs=2, space="PSUM") as psT,
            tc.tile_pool(name="psO", bufs=2, space="PSUM") as psO,
            tc.tile_pool(name="psD", bufs=1, space="PSUM") as psD,
            tc.tile_pool(name="psL", bufs=2, space="PSUM") as psL,
        ):
            # ---- load constants ----
            def load_const(t_in, shape, dtype=f32):
                t = cp.tile(shape, dtype, tag=t_in.name)
                nc.sync.dma_start(out=t[:], in_=t_in[:])
                return t

            xt0 = pp.tile([128, 2, NS], f32, tag="xt0")
            nc.sync.dma_start(out=xt0[:], in_=xt_in[:])
            idx16s = load_const(idxs_in, [128, icols], i16)
            dstl16 = load_const(dstl_in, [128, T * CH], f16)
            Wt = [
                load_const(w_in[0], [128, 2, 512]),
                load_const(w_in[1], [128, 4, 512]),
                load_const(w_in[2], [128, 4, 40]),
            ]
            At = [
                load_const(a_in[0], [128, 4, 16]),
                load_const(a_in[1], [128, 4, 16]),
                load_const(a_in[2], [40, 2]),
            ]
            Bt = [
                load_const(b_in[0], [128, 512]),
                load_const(b_in[1], [128, 512]),
                load_const(b_in[2], [128, 40]),
            ]
            iota16 = load_const(iota_in, [128, 128], f16)
            iotaP16 = load_const(iotap_in, [128, 1], f16)
            ident = load_const(ident_in, [128, 128])
            aldsb = pp.tile([TS, T, 8], f16, tag="aldsb")

            xt_cur = xt0
            for l, (fbi, fout, fbo, Hh, Cc, row) in enumerate(specs):
                # ============ phase A: hxT = W.T-contract(XT), logits ============
                if l < 2:
                    hxT = pp.tile([128, fbo, NS], f32, tag="hxT")
                else:
                    hxT = pp.tile([40, NS], f32, tag="hxT2")
                alT = pp.tile([16 if l < 2 else 2, NS], f32, tag="alT")

                # per-tile phase A so it can overlap the previous layer's
                # phase B (xt_next tile deps resolve tile by tile)
                mpart = 128 if l < 2 else 40
                napart = 16 if l < 2 else 2
                for t in range(T):
                    nsl = slice(t * TS, (t + 1) * TS)
                    for fo in range(fbo):
                        ps = psA.tile([mpart, TS], f32, tag="big")
                        for fb in range(fbi):
                            nc.tensor.matmul(
                                out=ps[:],
                                lhsT=Wt[l][:, fb, fo * 128 : fo * 128 + mpart],
                                rhs=xt_cur[:, fb, nsl],
                                start=(fb == 0),
                                stop=(fb == fbi - 1),
                            )
                        if l < 2:
                            nc.scalar.activation(
                                out=hxT[:, fo, nsl], in_=ps[:], func=Act.Copy
                            )
                        else:
                            nc.scalar.activation(
                                out=hxT[:, nsl], in_=ps[:], func=Act.Copy
                            )
                    ps2 = psA.tile([napart, TS], f32, tag="big")
                    if l < 2:
                        for fb in range(fbo):
                            nc.tensor.matmul(
                                out=ps2[:],
                                lhsT=At[l][:, fb, :napart],
                                rhs=hxT[:, fb, nsl],
                                start=(fb == 0),
                                stop=(fb == fbo - 1),
                            )
                    else:
                        nc.tensor.matmul(
                            out=ps2[:],
                            lhsT=At[l][:40, :2],
                            rhs=hxT[:40, nsl],
                            start=True,
                            stop=True,
                        )
                    nc.scalar.activation(out=alT[:, nsl], in_=ps2[:], func=Act.Copy)

                    # transpose to node-major fp16 rows; al_src fp32 in-row
                    stg = sb.tile([TS, row], f16, tag="stg")
                    if l < 2:
                        for fo in range(fbo):
                            pt = psT.tile([TS, 128], f32, tag="tr")
                            nc.tensor.transpose(
                                out=pt[:], in_=hxT[:, fo, nsl], identity=ident[:]
                            )
                            nc.scalar.activation(
                                out=stg[:, fo * 128 : (fo + 1) * 128],
                                in_=pt[:],
                                func=Act.Copy,
                            )
                        pa = psT.tile([TS, 16], f32, tag="tr")
                        nc.tensor.transpose(
                            out=pa[:], in_=alT[:16, nsl], identity=ident[:16, :16]
                        )
                        nc.scalar.activation(
                            out=stg[:, 512:528].bitcast(f32),
                            in_=pa[:, 0:8],
                            func=Act.Copy,
                        )
                        nc.scalar.activation(
                            out=aldsb[:, t, :], in_=pa[:, 8:16], func=Act.Copy
                        )
                        nc.vector.memset(stg[:, 528:row], 0.0)
                    else:
                        pt = psT.tile([TS, 40], f32, tag="tr")
                        nc.tensor.transpose(
                            out=pt[:], in_=hxT[:40, nsl], identity=ident[:40, :40]
                        )
                        nc.scalar.activation(out=stg[:, 0:40], in_=pt[:], func=Act.Copy)
                        pa = psT.tile([TS, 2], f32, tag="tr")
                        nc.tensor.transpose(
                            out=pa[:], in_=alT[:2, nsl], identity=ident[:2, :2]
                        )
                        nc.scalar.activation(
                            out=stg[:, 40:42].bitcast(f32),
                            in_=pa[:, 0:1],
                            func=Act.Copy,
                        )
                        nc.scalar.activation(
                            out=aldsb[:, t, 0:1], in_=pa[:, 1:2], func=Act.Copy
                        )
                        nc.vector.memset(stg[:, 42:row], 0.0)
                    nc.sync.dma_start(out=agin[l][nsl, :], in_=stg[:])

                nc.gpsimd.collective_compute(
                    "AllGather",
                    Alu.bypass,
                    replica_groups=groups,
                    ins=[agin[l][:]],
                    outs=[tbl[l][:]],
                )

                # ============ phase B: edge aggregation ============
                if l < 2:
                    xt_next = pp.tile([128, 4, NS], f32, tag=f"xt{l + 1}")
                hc0 = max(1, (CH + 2) // 3)  # chunks per gather batch
                for t in range(T):
                    # one-hot masks for the whole tile, one DVE op each
                    mk = sb.tile([128, CH, TS], f16, tag="mk")
                    nc.vector.tensor_tensor(
                        out=mk[:],
                        in0=bcast_mid(iota16[:, 0:TS], CH),
                        in1=dstl16[:, t * CH : (t + 1) * CH].to_broadcast(
                            [128, CH, TS]
                        ),
                        op=Alu.is_equal,
                    )
                    dT = sb.tile([128, CH * 128], f16, tag="dT")
                    nc.sync.dma_start(
                        out=dT[:],
                        in_=dstlT_in[:, t * CH * 128 : (t + 1) * CH * 128],
                    )
                    mkT = sb.tile([TS, CH * 128], f16, tag="mkT")
                    nc.vector.tensor_tensor(
                        out=mkT[:],
                        in0=bcast_col(iotaP16[:TS, 0:1], CH * 128),
                        in1=dT[:TS, :],
                        op=Alu.is_equal,
                    )
                    ps_out = psO.tile([TS, fout], f32, tag="out")
                    ps_den = psD.tile([TS, Hh], f32, tag="den")
                    jj = 0
                    while jj < CH:
                        hc = min(hc0, CH - jj)
                        ic0 = (t * CH + jj) * 8
                        G = sb3.tile([128, hc0, row], f16, tag="G")
                        nc.gpsimd.dma_gather(
                            out_ap=G[:, :hc, :],
                            in_ap=tbl[l][:],
                            idxs_ap=idx16s[:, ic0 : ic0 + hc * 8],
                            num_idxs=hc * 128,
                            num_idxs_reg=hc * 128,
                            elem_size=row,
                        )
                        # logits: leaky(als[src] + ald[dst]) in fp32, then exp
                        lg = sb.tile([128, hc0, Hh], f32, tag="lg")
                        alo = 512 if l < 2 else 40
                        ps_ald = psL.tile([128, hc0 * Hh], f32, tag="aldpe")
                        for j in range(hc):
                            nc.tensor.matmul(
                                out=ps_ald[:, j * Hh : (j + 1) * Hh],
                                lhsT=mkT[:, (jj + j) * 128 : (jj + j + 1) * 128],
                                rhs=aldsb[:, t, :Hh],
                                start=True,
                                stop=True,
                                skip_group_check=True,
                            )
                        nc.vector.tensor_tensor(
                            out=lg[:, :hc, :],
                            in0=G[:, :hc, alo : alo + 2 * Hh].bitcast(f32),
                            in1=ps_ald[:].rearrange("p (j h) -> p j h", h=Hh)[
                                :, :hc, :
                            ],
                            op=Alu.add,
                        )
                        # exp(leaky(x)) == max(exp(x), exp(0.2*x))
                        t2 = sb.tile([128, hc0, Hh], f16, tag="t2")
                        nc.scalar.activation(
                            out=t2[:, :hc, :], in_=lg[:, :hc, :], func=Act.Exp
                        )
                        ex = sb.tile([128, hc0, Hh], f16, tag="ex")
                        nc.scalar.activation(
                            out=ex[:, :hc, :],
                            in_=lg[:, :hc, :],
                            func=Act.Exp,
                            scale=NEG,
                        )
                        nc.vector.tensor_tensor(
                            out=ex[:, :hc, :],
                            in0=ex[:, :hc, :],
                            in1=t2[:, :hc, :],
                            op=Alu.max,
                        )
                        # scale features by exp(logit) per (edge, head)
                        G2 = sb3.tile([128, hc0, Hh, Cc], f16, tag="G2")
                        nc.vector.tensor_tensor(
                            out=G2[:, :hc, :, :],
                            in0=G[:, :hc, 0:fout].rearrange(
                                "p j (h c) -> p j h c", h=Hh
                            ),
                            in1=ex[:, :hc, :].to_broadcast([128, hc, Hh, Cc]),
                            op=Alu.mult,
                        )
                        for j in range(hc):
                            glob_j = jj + j
                            nc.tensor.matmul(
                                out=ps_out[:],
                                lhsT=mk[:, glob_j, :],
                                rhs=G2[:, j],
                                start=(glob_j == 0),
                                stop=(glob_j == CH - 1),
                                skip_group_check=True,
                            )
                            nc.tensor.matmul(
                                out=ps_den[:],
                                lhsT=mk[:, glob_j, :],
                                rhs=ex[:, j],
                                start=(glob_j == 0),
                                stop=(glob_j == CH - 1),
                                skip_group_check=True,
                            )
                        jj += hc
                    # tail: divide by denominator, bias, activation
                    den = sb.tile([TS, Hh], f32, tag="dent")
                    nc.vector.tensor_scalar(
                        out=den[:], in0=ps_den[:], scalar1=1e-16, scalar2=None,
                        op0=Alu.add,
                    )
                    rcp = sb.tile([TS, Hh], f32, tag="rcp")
                    nc.vector.reciprocal(out=rcp[:], in_=den[:])
                    y = sb.tile([TS, fout], f32, tag="y")
                    nc.vector.tensor_tensor(
                        out=y[:].rearrange("p (h c) -> p h c", h=Hh),
                        in0=ps_out[:].rearrange("p (h c) -> p h c", h=Hh),
                        in1=rcp[:].to_broadcast([TS, Hh, Cc]),
                        op=Alu.mult,
                    )
                    nc.vector.tensor_tensor(
                        out=y[:], in0=y[:], in1=Bt[l][:TS, :fout], op=Alu.add
                    )
                    nsl = slice(t * TS, (t + 1) * TS)
                    if l < 2:
                        # elu(y) = max(y,0) + exp(min(y,0)) - 1
                        ymin = sb.tile([TS, fout], f32, tag="ymin")
                        nc.vector.tensor_scalar(
                            out=ymin[:], in0=y[:], scalar1=0.0, scalar2=None,
                            op0=Alu.min,
                        )
                        eneg = sb.tile([TS, fout], f32, tag="eneg")
                        nc.scalar.activation(out=eneg[:], in_=ymin[:], func=Act.Exp)
                        nc.vector.tensor_scalar(
                            out=eneg[:], in0=eneg[:], scalar1=1.0, scalar2=None,
                            op0=Alu.subtract,
                        )
                        x2 = sb.tile([TS, fout], f32, tag="x2")
                        nc.vector.tensor_scalar(
                            out=x2[:], in0=y[:], scalar1=0.0, scalar2=None,
                            op0=Alu.max,
                        )
                        nc.vector.tensor_tensor(
                            out=x2[:], in0=x2[:], in1=eneg[:], op=Alu.add
                        )
                        # transpose into next layer's feature-major XT
                        for fo in range(4):
                            pt = psT.tile([128, TS], f32, tag="tr")
                            nc.tensor.transpose(
                                out=pt[:],
                                in_=x2[:, fo * 128 : (fo + 1) * 128],
                                identity=ident[:TS, :TS],
                            )
                            nc.scalar.activation(
                                out=xt_next[:, fo, nsl], in_=pt[:], func=Act.Copy
                            )
                    else:
                        nc.sync.dma_start(out=out_ext[nsl, :], in_=y[:, :NUM_CLASSES])
                if l < 2:
                    xt_cur = xt_next

    nc.compile()
    return nc


def _make_in_maps(inputs, CH, idx16s, dl16, dlT16):
    x = np.asarray(inputs["x"], np.float32)
    iota16 = np.tile(np.arange(128, dtype=np.float16), (128, 1))
    ident = np.eye(128, dtype=np.float32)
    common = {
        "W0": _feat_major(np.asarray(inputs["W0"], np.float32), 2),
        "W1": _feat_major(np.asarray(inputs["W1"], np.float32), 4),
        "W2": _feat_major(np.asarray(inputs["W2"], np.float32), 4),
        "A0": _feat_major(
            _block_diag_a(np.asarray(inputs["a_src0"]), np.asarray(inputs["a_dst0"])), 4
        ),
        "A1": _feat_major(
            _block_diag_a(np.asarray(inputs["a_src1"]), np.asarray(inputs["a_dst1"])), 4
        ),
        "A2": np.ascontiguousarray(
            np.stack(
                [
                    np.asarray(inputs["a_src2"], np.float32)[0],
                    np.asarray(inputs["a_dst2"], np.float32)[0],
                ],
                axis=1,
            )
        ),
        "B0": np.tile(np.asarray(inputs["b0"], np.float32), (128, 1)),
        "B1": np.tile(np.asarray(inputs["b1"], np.float32), (128, 1)),
        "B2": np.tile(np.asarray(inputs["b2"], np.float32), (128, 1)),
        "iota16": iota16,
        "iotaP16": np.arange(128, dtype=np.float16).reshape(128, 1),
        "ident": ident,
    }
    in_maps = []
    for k in range(NCORES):
        xs = x[k * NS : (k + 1) * NS]  # [NS, 256]
        xt = np.ascontiguousarray(xs.T.reshape(2, 128, NS).transpose(1, 0, 2))
        in_maps.append(
            dict(
                common,
                xt=xt,
                idx16s=idx16s[k],
                dstl16=dl16[k],
                dstlT16=np.ascontiguousarray(dlT16[k]),
            )
        )
    return in_maps


def get_program_and_maps(inputs):
    CH, idx16s, dl16, dlT16 = _preprocess(np.asarray(inputs["edge_index"]))
    if CH not in _cache:
        _cache[CH] = _build_program(CH)
    nc = _cache[CH]
    return nc, _make_in_maps(inputs, CH, idx16s, dl16, dlT16)


def kernel(**inputs):
    from concourse.bass_utils import run_bass_kernel_spmd

    nc, in_maps = get_program_and_maps(inputs)
    res = run_bass_kernel_spmd(nc, in_maps, list(range(NCORES)))
    outs = [res.results[k]["out"] for k in range(NCORES)]
    return np.concatenate(outs, axis=0)


# revision 20
# speedup vs baseline: 2.1656x; 1.1498x over previous
"""GAT (3-layer, PyG-style) on 8 Trainium2 NeuronCores via Bass/Tile.

Sharding: edges are partitioned by dst-node range (1250 nodes per core).
Per layer: (A) node-parallel feature projection + attention logits;
AllGather of a node-major fp16 row table [hx_fp16 | al_src_fp32 | pad]
into each core's DRAM (al_dst stays in core-local SBUF); (B)
edge-parallel aggregation: hardware dma_gather of per-edge rows by src
id; al_dst broadcast to edges via a transposed one-hot matmul on the
tensor engine; exp(leaky(logits)) scaling on fp32 logits; segment-sum
via one-hot-mask matmuls accumulated in PSUM (the softmax denominator
rides a second matmul; division is applied per dst node afterwards).
The softmax max-subtraction is skipped: logits are bounded by
construction (|e| < ~8) so exp stays well inside fp16/fp32 range.
"""

import sys

sys.path.insert(0, "/opt/trn_rl_repo")

import numpy as np

N = 10000
E = 160000
NCORES = 8
NS = 1250          # nodes per core
T = 10             # dst tiles per core
TS = 125           # nodes per dst tile
NEG = 0.2          # leaky_relu slope

F_IN = 256
HC = 512
NUM_CLASSES = 40

_cache = {}


def _wrap16(a):
    """[T, CH, 128] per-slot values -> [128, T*CH*8] int16 wrapped layout.

    dma_gather consumes index i from (partition i%16, col i//16), replicated
    across the eight 16-partition groups. Slot (t, j, q) is flat index
    i = j*128 + q within tile t's column block.
    """
    Tn, CHn, _ = a.shape
    b = a.reshape(Tn * CHn * 8, 16).astype(np.int16)
    m = np.ascontiguousarray(b.T)          # [16, T*CH*8]
    return np.tile(m, (8, 1))              # [128, T*CH*8]


def _preprocess(edge_index):
    """Group edges (incl. self-loops) by (core, dst-tile); pad chunks of 128.

    Padding slots keep src=0 (harmless gather) and mask value 127 (>= TS)
    so one-hot mask rows are all-zero and they contribute nothing.
    """
    src = np.concatenate([np.asarray(edge_index[0]), np.arange(N)]).astype(np.int64)
    dst = np.concatenate([np.asarray(edge_index[1]), np.arange(N)]).astype(np.int64)
    gtile = dst // TS                       # global tile id 0..79
    order = np.argsort(gtile, kind="stable")
    src, dst, gtile = src[order], dst[order], gtile[order]
    counts = np.bincount(gtile, minlength=NCORES * T)
    CH = int(np.ceil(counts.max() / 128))
    starts = np.concatenate([[0], np.cumsum(counts)])

    idx16s = np.empty((NCORES, 128, T * CH * 8), np.int16)
    dl16 = np.empty((NCORES, 128, T * CH), np.float16)
    dlT16 = np.empty((NCORES, 128, T * CH * 128), np.float16)
    for k in range(NCORES):
        sa = np.zeros((T, CH, 128), np.int64)
        dl = np.full((T, CH, 128), 127.0, np.float32)
        for t in range(T):
            g = k * T + t
            s0, s1 = starts[g], starts[g + 1]
            m = s1 - s0
            i = np.arange(m)
            js, qs = i // 128, i % 128
            sa[t, js, qs] = src[s0:s1]
            dl[t, js, qs] = (dst[s0:s1] % TS).astype(np.float32)
        idx16s[k] = _wrap16(sa)
        # mask layout: value at (p, t*CH + j) = dl[t, j, p]
        dl16[k] = dl.transpose(2, 0, 1).reshape(128, T * CH).astype(np.float16)
        # maskT layout: value at (p, (t*CH+j)*128 + q) = dl[t, j, q], any p
        dlT16[k] = np.broadcast_to(
            dl.reshape(1, T * CH * 128), (128, T * CH * 128)
        ).astype(np.float16)
    return CH, idx16s, dl16, dlT16


def _feat_major(w, fb):
    """[K, M] -> [128, fb, M] with element (p, b, m) = w[b*128 + p, m]."""
    K, M = w.shape
    assert K == fb * 128
    return np.ascontiguousarray(w.reshape(fb, 128, M).transpose(1, 0, 2))


def _block_diag_a(a_src, a_dst):
    """[H, C] pair -> [H*C, 16] block-diag (cols 0:8 src, 8:16 dst)."""
    h, c = a_src.shape
    blk = np.zeros((h * c, 16), np.float32)
    for i in range(h):
        blk[i * c : (i + 1) * c, i] = a_src[i]
        blk[i * c : (i + 1) * c, 8 + i] = a_dst[i]
    return blk


def _build_program(CH):
    import concourse.bass as bass
    import concourse.mybir as mybir
    import concourse.bacc as bacc
    import concourse.tile as tile

    f32 = mybir.dt.float32
    f16 = mybir.dt.float16
    i16 = mybir.dt.int16
    Alu = mybir.AluOpType
    Act = mybir.ActivationFunctionType

    def bcast_mid(ap, n, axis=1):
        """Insert a stride-0 dim of extent n at `axis` of an AP."""
        newap = [list(d) for d in ap.ap]
        newap.insert(axis, [0, n])
        return bass.AP(ap.tensor, ap.offset, newap)

    def bcast_col(ap2d, n):
        """[P, 1] AP -> [P, n] with stride-0 free dim."""
        return bass.AP(ap2d.tensor, ap2d.offset, [list(ap2d.ap[0]), [0, n]])

    nc = bacc.Bacc(
        "TRN2",
        target_bir_lowering=False,
        debug=False,
        enable_asserts=False,
        num_devices=NCORES,
    )

    icols = T * CH * 8
    NSL = [(0, 512), (512, 512), (1024, 226)]  # 1250 split for matmul N<=512

    # layer specs: (fin_blocks, fout, fout_blocks, H, C, row_f16)
    # row_f16: gathered-table row length in fp16 units (256B-aligned)
    specs = [
        (2, 512, 4, 8, 64, 640),
        (4, 512, 4, 8, 64, 640),
        (4, 40, 1, 1, 40, 128),
    ]

    # ---- external I/O ----
    xt_in = nc.dram_tensor("xt", [128, 2, NS], f32, kind="ExternalInput")
    idxs_in = nc.dram_tensor("idx16s", [128, icols], i16, kind="ExternalInput")
    dstl_in = nc.dram_tensor("dstl16", [128, T * CH], f16, kind="ExternalInput")
    dstlT_in = nc.dram_tensor(
        "dstlT16", [128, T * CH * 128], f16, kind="ExternalInput"
    )
    w_in = [
        nc.dram_tensor("W0", [128, 2, 512], f32, kind="ExternalInput"),
        nc.dram_tensor("W1", [128, 4, 512], f32, kind="ExternalInput"),
        nc.dram_tensor("W2", [128, 4, 40], f32, kind="ExternalInput"),
    ]
    a_in = [
        nc.dram_tensor("A0", [128, 4, 16], f32, kind="ExternalInput"),
        nc.dram_tensor("A1", [128, 4, 16], f32, kind="ExternalInput"),
        nc.dram_tensor("A2", [40, 2], f32, kind="ExternalInput"),
    ]
    b_in = [
        nc.dram_tensor("B0", [128, 512], f32, kind="ExternalInput"),
        nc.dram_tensor("B1", [128, 512], f32, kind="ExternalInput"),
        nc.dram_tensor("B2", [128, 40], f32, kind="ExternalInput"),
    ]
    iota_in = nc.dram_tensor("iota16", [128, 128], f16, kind="ExternalInput")
    iotap_in = nc.dram_tensor("iotaP16", [128, 1], f16, kind="ExternalInput")
    ident_in = nc.dram_tensor("ident", [128, 128], f32, kind="ExternalInput")
    out_ext = nc.dram_tensor("out", [NS, NUM_CLASSES], f32, kind="ExternalOutput")

    # internal DRAM
    agin = [
        nc.dram_tensor(f"agin{l}", [NS, specs[l][5]], f16, kind="Internal")
        for l in range(3)
    ]
    tbl = [
        nc.dram_tensor(
            f"tbl{l}", [N, specs[l][5]], f16, kind="Internal", addr_space="Shared"
        )
        for l in range(3)
    ]

    groups = [list(range(NCORES))]

    with tile.TileContext(nc) as tc:
        with (
            tc.tile_pool(name="const", bufs=1) as cp,
            tc.tile_pool(name="sb", bufs=2) as sb,
            tc.tile_pool(name="sb3", bufs=3) as sb3,
            tc.tile_pool(name="sb4", bufs=4) as sb4,
            tc.tile_pool(name="persist", bufs=1) as pp,
            tc.tile_pool(name="psA", bufs=2, space="PSUM") as psA,
            tc.tile_pool(name="psT", bufs=2, space="PSUM") as psT,
            tc.tile_pool(name="psO", bufs=2, space="PSUM") as psO,
            tc.tile_pool(name="psD", bufs=1, space="PSUM") as psD,
            tc.tile_pool(name="psL", bufs=1, space="PSUM") as psL,
        ):
            # ---- load constants ----
            def load_const(t_in, shape, dtype=f32):
                t = cp.tile(shape, dtype, tag=t_in.name)
                nc.sync.dma_start(out=t[:], in_=t_in[:])
                return t

            xt0 = pp.tile([128, 2, NS], f32, tag="xt0")
            nc.sync.dma_start(out=xt0[:], in_=xt_in[:])
            idx16s = load_const(idxs_in, [128, icols], i16)
            dstl16 = load_const(dstl_in, [128, T * CH], f16)
            Wt = [
                load_const(w_in[0], [128, 2, 512]),
                load_const(w_in[1], [128, 4, 512]),
                load_const(w_in[2], [128, 4, 40]),
            ]
            At = [
                load_const(a_in[0], [128, 4, 16]),
                load_const(a_in[1], [128, 4, 16]),
                load_const(a_in[2], [40, 2]),
            ]
            Bt = [
                load_const(b_in[0], [128, 512]),
                load_const(b_in[1], [128, 512]),
                load_const(b_in[2], [128, 40]),
            ]
            iota16 = load_const(iota_in, [128, 128], f16)
            iotaP16 = load_const(iotap_in, [128, 1], f16)
            ident = load_const(ident_in, [128, 128])
            aldsb = pp.tile([TS, T, 8], f16, tag="aldsb")

            xt_cur = xt0
            for l, (fbi, fout, fbo, Hh, Cc, row) in enumerate(specs):
                # ============ phase A: hxT = W.T-contract(XT), logits ============
                if l < 2:
                    hxT = pp.tile([128, fbo, NS], f32, tag="hxT")
                else:
                    hxT = pp.tile([40, NS], f32, tag="hxT2")
                alT = pp.tile([16 if l < 2 else 2, NS], f32, tag="alT")

                # per-tile phase A so it can overlap the previous layer's
                # phase B (xt_next tile deps resolve tile by tile)
                mpart = 128 if l < 2 else 40
                napart = 16 if l < 2 else 2
                for t in range(T):
                    nsl = slice(t * TS, (t + 1) * TS)
                    for fo in range(fbo):
                        ps = psA.tile([mpart, TS], f32, tag="big")
                        for fb in range(fbi):
                            nc.tensor.matmul(
                                out=ps[:],
                                lhsT=Wt[l][:, fb, fo * 128 : fo * 128 + mpart],
                                rhs=xt_cur[:, fb, nsl],
                                start=(fb == 0),
                                stop=(fb == fbi - 1),
                            )
                        if l < 2:
                            nc.scalar.activation(
                                out=hxT[:, fo, nsl], in_=ps[:], func=Act.Copy
                            )
                        else:
                            nc.scalar.activation(
                                out=hxT[:, nsl], in_=ps[:], func=Act.Copy
                            )
                    ps2 = psA.tile([napart, TS], f32, tag="big")
                    if l < 2:
                        for fb in range(fbo):
                            nc.tensor.matmul(
                                out=ps2[:],
                                lhsT=At[l][:, fb, :napart],
                                rhs=hxT[:, fb, nsl],
                                start=(fb == 0),
                                stop=(fb == fbo - 1),
                            )
                    else:
                        nc.tensor.matmul(
                            out=ps2[:],
                            lhsT=At[l][:40, :2],
                            rhs=hxT[:40, nsl],
                            start=True,
                            stop=True,
                        )
                    nc.scalar.activation(out=alT[:, nsl], in_=ps2[:], func=Act.Copy)

                    # transpose to node-major fp16 rows; al_src fp32 in-row
                    stg = sb.tile([TS, row], f16, tag="stg")
                    if l < 2:
                        for fo in range(fbo):
                            pt = psT.tile([TS, 128], f32, tag="tr")
                            nc.tensor.transpose(
                                out=pt[:], in_=hxT[:, fo, nsl], identity=ident[:]
                            )
                            nc.scalar.activation(
                                out=stg[:, fo * 128 : (fo + 1) * 128],
                                in_=pt[:],
                                func=Act.Copy,
                            )
                        pa = psT.tile([TS, 16], f32, tag="tr")
                        nc.tensor.transpose(
                            out=pa[:], in_=alT[:16, nsl], identity=ident[:16, :16]
                        )
                        nc.scalar.activation(
                            out=stg[:, 512:528].bitcast(f32),
                            in_=pa[:, 0:8],
                            func=Act.Copy,
                        )
                        nc.scalar.activation(
                            out=aldsb[:, t, :], in_=pa[:, 8:16], func=Act.Copy
                        )
                        nc.vector.memset(stg[:, 528:row], 0.0)
                    else:
                        pt = psT.tile([TS, 40], f32, tag="tr")
                        nc.tensor.transpose(
                            out=pt[:], in_=hxT[:40, nsl], identity=ident[:40, :40]
                        )
                        nc.scalar.activation(out=stg[:, 0:40], in_=pt[:], func=Act.Copy)
                        pa = psT.tile([TS, 2], f32, tag="tr")
                        nc.tensor.transpose(
                            out=pa[:], in_=alT[:2, nsl], identity=ident[:2, :2]
                        )
                        nc.scalar.activation(
                            out=stg[:, 40:42].bitcast(f32),
                            in_=pa[:, 0:1],
                            func=Act.Copy,
                        )
                        nc.scalar.activation(
                            out=aldsb[:, t, 0:1], in_=pa[:, 1:2], func=Act.Copy
                        )
                        nc.vector.memset(stg[:, 42:row], 0.0)
                    nc.sync.dma_start(out=agin[l][nsl, :], in_=stg[:])

                nc.gpsimd.collective_compute(
                    "AllGather",
                    Alu.bypass,
                    replica_groups=groups,
                    ins=[agin[l][:]],
                    outs=[tbl[l][:]],
                )

                # ============ phase B: edge aggregation ============
                if l < 2:
                    xt_next = pp.tile([128, 4, NS], f32, tag=f"xt{l + 1}")
                hc0 = max(1, (CH + 2) // 3)  # chunks per gather batch
                for t in range(T):
                    # one-hot masks for the whole tile, one DVE op each
                    mk = sb.tile([128, CH, TS], f16, tag="mk")
                    nc.vector.tensor_tensor(
                        out=mk[:],
                        in0=bcast_mid(iota16[:, 0:TS], CH),
                        in1=dstl16[:, t * CH : (t + 1) * CH].to_broadcast(
                            [128, CH, TS]
                        ),
                        op=Alu.is_equal,
                    )
                    dT = sb.tile([128, CH * 128], f16, tag="dT")
                    nc.sync.dma_start(
                        out=dT[:],
                        in_=dstlT_in[:, t * CH * 128 : (t + 1) * CH * 128],
                    )
                    mkT = sb.tile([TS, CH * 128], f16, tag="mkT")
                    nc.vector.tensor_tensor(
                        out=mkT[:],
                        in0=bcast_col(iotaP16[:TS, 0:1], CH * 128),
                        in1=dT[:TS, :],
                        op=Alu.is_equal,
                    )
                    ps_out = psO.tile([TS, fout], f32, tag="out")
                    ps_den = psD.tile([TS, Hh], f32, tag="den")
                    jj = 0
                    while jj < CH:
                        hc = min(hc0, CH - jj)
                        ic0 = (t * CH + jj) * 8
                        G = sb4.tile([128, hc0, row], f16, tag="G")
                        nc.gpsimd.dma_gather(
                            out_ap=G[:, :hc, :],
                            in_ap=tbl[l][:],
                            idxs_ap=idx16s[:, ic0 : ic0 + hc * 8],
                            num_idxs=hc * 128,
                            num_idxs_reg=hc * 128,
                            elem_size=row,
                        )
                        # logits: leaky(als[src] + ald[dst]) in fp32, then exp
                        lg = sb3.tile([128, hc0, Hh], f32, tag="lg")
                        alo = 512 if l < 2 else 40
                        ps_ald = psL.tile([128, hc0 * Hh], f32, tag="aldpe")
                        for j in range(hc):
                            nc.tensor.matmul(
                                out=ps_ald[:, j * Hh : (j + 1) * Hh],
                                lhsT=mkT[:, (jj + j) * 128 : (jj + j + 1) * 128],
                                rhs=aldsb[:, t, :Hh],
                                start=True,
                                stop=True,
                                skip_group_check=True,
                            )
                        nc.vector.tensor_tensor(
                            out=lg[:, :hc, :],
                            in0=G[:, :hc, alo : alo + 2 * Hh].bitcast(f32),
                            in1=ps_ald[:].rearrange("p (j h) -> p j h", h=Hh)[
                                :, :hc, :
                            ],
                            op=Alu.add,
                        )
                        # exp(leaky(x)) == max(exp(x), exp(0.2*x))
                        t2 = sb3.tile([128, hc0, Hh], f16, tag="t2")
                        nc.scalar.activation(
                            out=t2[:, :hc, :], in_=lg[:, :hc, :], func=Act.Exp
                        )
                        ex = sb3.tile([128, hc0, Hh], f16, tag="ex")
                        nc.scalar.activation(
                            out=ex[:, :hc, :],
                            in_=lg[:, :hc, :],
                            func=Act.Exp,
                            scale=NEG,
                        )
                        nc.vector.tensor_tensor(
                            out=ex[:, :hc, :],
                            in0=ex[:, :hc, :],
                            in1=t2[:, :hc, :],
                            op=Alu.max,
                        )
                        # scale features by exp(logit) per (edge, head)
                        G2 = sb3.tile([128, hc0, Hh, Cc], f16, tag="G2")
                        nc.vector.tensor_tensor(
                            out=G2[:, :hc, :, :],
                            in0=G[:, :hc, 0:fout].rearrange(
                                "p j (h c) -> p j h c", h=Hh
                            ),
                            in1=ex[:, :hc, :].to_broadcast([128, hc, Hh, Cc]),
                            op=Alu.mult,
                        )
                        for j in range(hc):
                            glob_j = jj + j
                            nc.tensor.matmul(
                                out=ps_out[:],
                                lhsT=mk[:, glob_j, :],
                                rhs=G2[:, j],
                                start=(glob_j == 0),
                                stop=(glob_j == CH - 1),
                                skip_group_check=True,
                            )
                            nc.tensor.matmul(
                                out=ps_den[:],
                                lhsT=mk[:, glob_j, :],
                                rhs=ex[:, j],
                                start=(glob_j == 0),
                                stop=(glob_j == CH - 1),
                                skip_group_check=True,
                            )
                        jj += hc
                    # tail: divide by denominator, bias, activation
                    den = sb.tile([TS, Hh], f32, tag="dent")
                    nc.vector.tensor_scalar(
                        out=den[:], in0=ps_den[:], scalar1=1e-16, scalar2=None,
                        op0=Alu.add,
                    )
                    rcp = sb.tile([TS, Hh], f32, tag="rcp")
                    nc.vector.reciprocal(out=rcp[:], in_=den[:])
                    y = sb.tile([TS, fout], f32, tag="y")
                    nc.vector.tensor_tensor(
                        out=y[:].rearrange("p (h c) -> p h c", h=Hh),
                        in0=ps_out[:].rearrange("p (h c) -> p h c", h=Hh),
                        in1=rcp[:].to_broadcast([TS, Hh, Cc]),
                        op=Alu.mult,
                    )
                    nc.vector.tensor_tensor(
                        out=y[:], in0=y[:], in1=Bt[l][:TS, :fout], op=Alu.add
                    )
                    nsl = slice(t * TS, (t + 1) * TS)
                    if l < 2:
                        # elu(y) = max(y,0) + exp(min(y,0)) - 1
                        ymin = sb.tile([TS, fout], f32, tag="ymin")
                        nc.vector.tensor_scalar(
                            out=ymin[:], in0=y[:], scalar1=0.0, scalar2=None,
                            op0=Alu.min,
                        )
                        eneg = sb.tile([TS, fout], f32, tag="eneg")
                        nc.scalar.activation(out=eneg[:], in_=ymin[:], func=Act.Exp)
                        nc.vector.tensor_scalar(
                            out=eneg[:], in0=eneg[:], scalar1=1.0, scalar2=None,
                            op0=Alu.subtract,
                        )
                        x2 = sb.tile([TS, fout], f32, tag="x2")
                        nc.vector.tensor_scalar(
                            out=x2[:], in0=y[:], scalar1=0.0, scalar2=None,
                            op0=Alu.max,
                        )
                        nc.vector.tensor_tensor(
                            out=x2[:], in0=x2[:], in1=eneg[:], op=Alu.add
                        )
                        # transpose into next layer's feature-major XT
                        for fo in range(4):
                            pt = psT.tile([128, TS], f32, tag="tr")
                            nc.tensor.transpose(
                                out=pt[:],
                                in_=x2[:, fo * 128 : (fo + 1) * 128],
                                identity=ident[:TS, :TS],
                            )
                            nc.scalar.activation(
                                out=xt_next[:, fo, nsl], in_=pt[:], func=Act.Copy
                            )
                    else:
                        nc.sync.dma_start(out=out_ext[nsl, :], in_=y[:, :NUM_CLASSES])
                if l < 2:
                    xt_cur = xt_next

    nc.compile()
    return nc


def _make_in_maps(inputs, CH, idx16s, dl16, dlT16):
    x = np.asarray(inputs["x"], np.float32)
    iota16 = np.tile(np.arange(128, dtype=np.float16), (128, 1))
    ident = np.eye(128, dtype=np.float32)
    common = {
        "W0": _feat_major(np.asarray(inputs["W0"], np.float32), 2),
        "W1": _feat_major(np.asarray(inputs["W1"], np.float32), 4),
        "W2": _feat_major(np.asarray(inputs["W2"], np.float32), 4),
        "A0": _feat_major(
            _block_diag_a(np.asarray(inputs["a_src0"]), np.asarray(inputs["a_dst0"])), 4
        ),
        "A1": _feat_major(
            _block_diag_a(np.asarray(inputs["a_src1"]), np.asarray(inputs["a_dst1"])), 4
        ),
        "A2": np.ascontiguousarray(
            np.stack(
                [
                    np.asarray(inputs["a_src2"], np.float32)[0],
                    np.asarray(inputs["a_dst2"], np.float32)[0],
                ],
                axis=1,
            )
        ),
        "B0": np.tile(np.asarray(inputs["b0"], np.float32), (128, 1)),
        "B1": np.tile(np.asarray(inputs["b1"], np.float32), (128, 1)),
        "B2": np.tile(np.asarray(inputs["b2"], np.float32), (128, 1)),
        "iota16": iota16,
        "iotaP16": np.arange(128, dtype=np.float16).reshape(128, 1),
        "ident": ident,
    }
    in_maps = []
    for k in range(NCORES):
        xs = x[k * NS : (k + 1) * NS]  # [NS, 256]
        xt = np.ascontiguousarray(xs.T.reshape(2, 128, NS).transpose(1, 0, 2))
        in_maps.append(
            dict(
                common,
                xt=xt,
                idx16s=idx16s[k],
                dstl16=dl16[k],
                dstlT16=np.ascontiguousarray(dlT16[k]),
            )
        )
    return in_maps


def get_program_and_maps(inputs):
    CH, idx16s, dl16, dlT16 = _preprocess(np.asarray(inputs["edge_index"]))
    if CH not in _cache:
        _cache[CH] = _build_program(CH)
    nc = _cache[CH]
    return nc, _make_in_maps(inputs, CH, idx16s, dl16, dlT16)


def kernel(**inputs):
    from concourse.bass_utils import run_bass_kernel_spmd

    nc, in_maps = get_program_and_maps(inputs)
    res = run_bass_kernel_spmd(nc, in_maps, list(range(NCORES)))
    outs = [res.results[k]["out"] for k in range(NCORES)]
    return np.concatenate(outs, axis=0)
